# revision 1
# baseline (speedup 1.0000x reference)
"""Trainium2 Bass kernel for nn_ConvexMLPBlock.

Reference computation (B=64, HW=196, D=768, E=256, C=10):
    S[b,h,e]  = (x[b,h,:] @ ag_w[e,:] + ag_b[e]) > 0          (sign patterns)
    z[b,h,p]  = x[b,h,:] @ lm_w[p,:]        (p = e*C + c)
    preds[b,c] = sum_{h,e} S[b,h,e] * z[b,h,e,c] / (HW*E)

Restructured to avoid materializing z (49 GFLOP -> ~10 GFLOP):
    G_b[e,d]   = sum_h S[b,h,e] * x[b,h,d]                    (per-batch masked moment)
    preds[b,c] = (1/(HW*E)) * sum_{e,d} G_b[e,d] * W[e,c,d]   (W = lm_w.reshape(E,C,D))

Sharding: data-parallel over B across the 8 NeuronCores (8 batches/core);
host concatenates the per-core (8,10) outputs.

Per-core pipeline:
    mm1: S^T[e,t] over all local tokens t=(b,h), PE matmul, K=d.
         "fp16x3" mode: x and ag_w are split hi/lo in fp16 on the host
         (x = hi + lo exactly captures ~22 mantissa bits); 3 matmul passes
         (hi*hi + hi*lo + lo*hi) give fp32-grade sign pre-activations at
         3 PE-cycles/row instead of fp32's 4, with fp16 fast weight loads.
    threshold: DVE tensor_scalar (psum + bias) > 0 -> 1.0/0.0 (bf16, exact)
    PE-transpose S^T -> S natural (h on partitions) per batch (bf16)
    mm2: G^T_b[d,e] contraction over h; f32r mode runs at 1 cyc/row
         (S is exactly representable; only x is rounded -> ~8e-5 rel err)
    final: 1536 accumulating PE matmuls (K=128 chunks of (d,e)) into one
           [10,8] PSUM tile, fp16 operands (1 cyc/row at N=8); the 1/(HW*E)
           scale is applied on the output copy to avoid fp16 underflow.
"""

import numpy as np

import concourse.bass as bass
import concourse.mybir as mybir
import concourse.tile as tile
from concourse.tile import add_dep_helper
from concourse.bass_utils import run_bass_kernel_spmd

# Problem constants (hardcoded per contract).
B = 64
HW = 196
D = 768
E = 256
C = 10
NCORES = 8
BL = B // NCORES          # local batches per core
T = BL * HW               # local tokens = 1568
KT = D // 128             # 6 d-tiles
ET = E // 128             # 2 e-tiles
W1 = 392                  # mm1 moving-dim chunk (4 chunks of 392 = 1568)
NCH = T // W1

FP32 = mybir.dt.float32
F32R = mybir.dt.float32r
BF16 = mybir.dt.bfloat16
FP16 = mybir.dt.float16


def _patched_drain_and_barrier(self, tick_clock, wait_clock):
    """This toolchain's walrus rejects >1 sync-wait on CTRL-class (Drain)
    instructions. Split the tail drain's global-clock waits across multiple
    single-wait drains. Semantics preserved: SP observes every DMA-queue
    semaphore before the all-engine barrier."""
    drain_inst = self.nc.sync.drain()
    wait_clock.add_sem_waits(
        drain_inst.ins, tile.ScopedClock({None: tick_clock.global_clock})
    )
    si = drain_inst.ins.sync_info
    if si is not None and si.on_wait is not None and len(si.on_wait) > 1:
        waits = list(si.on_wait)
        drain_inst.ins.sync_info = mybir.SyncInfo(
            on_wait=[waits[0]], on_update=list(si.on_update or [])
        )
        for w in waits[1:]:
            extra = self.nc.sync.drain()
            extra.ins.sync_info = mybir.SyncInfo(on_wait=[w], on_update=[])

    self.nc.all_engine_barrier()
    assert self.sems is not None
    popped = self.nc._tile_sem_poison_stack.pop()
    assert popped is self._sem_poison
    self.nc.clear_and_free_semaphores(list(self.sems.allocated().values()))
    self.nc.all_engine_barrier()


tile.TileContext._drain_and_barrier = _patched_drain_and_barrier


def _split_multiwait_json(bj: bytes) -> bytes:
    """Walrus in this toolchain accepts at most one sync-wait per instruction.
    For any instruction with N>1 waits, hoist N-1 waits onto same-engine NoOps
    inserted immediately before it. Engines execute program-order, so for
    compute instructions this is semantically identical; for DMAs it
    conservatively blocks the issuing engine instead of the queue."""
    import json

    m = json.loads(bj)
    changed = False
    for fn in m["functions"]:
        for bb in fn["blocks"]:
            new_insts = []
            for inst in bb["instructions"]:
                si = inst.get("sync_info")
                ow = (si or {}).get("on_wait") or []
                if len(ow) > 1:
                    for j, w in enumerate(ow[:-1]):
                        new_insts.append(
                            {
                                "name": f"{inst['name']}__w{j}",
                                "opcode": "NoOp",
                                "engine": inst["engine"],
                                "ins": [],
                                "outs": [],
                                "sync_info": {"on_update": [], "on_wait": [w]},
                            }
                        )
                    si["on_wait"] = [ow[-1]]
                    changed = True
                new_insts.append(inst)
            bb["instructions"] = new_insts
    if not changed:
        return bj
    return json.dumps(m).encode()


_orig_to_json_bytes = bass.Bass.to_json_bytes


def _patched_to_json_bytes(self, *a, **k):
    return _split_multiwait_json(_orig_to_json_bytes(self, *a, **k))


bass.Bass.to_json_bytes = _patched_to_json_bytes


EG = 16              # e's per final-stage group
NG = E // EG         # 16 groups per dt


def build_program(mm1="fp16x3", mm2="fp16", fin="fp16",
                  phases=("mm1", "tr", "mm2", "fin")):
    assert mm1 in ("fp32", "fp16x3")
    assert mm2 in ("fp32", "fp16")
    assert fin in ("fp16",)
    DT2 = FP16 if mm2 == "fp16" else FP32   # mm2 operand dtype (xn, sn)

    nc = bass.Bass()

    if mm1 == "fp32":
        xt_d = [nc.dram_tensor("xt", (D, T), FP32, kind="ExternalInput").ap()]
        agt_d = [nc.dram_tensor("agt", (D, E), FP32, kind="ExternalInput").ap()]
        DTX = FP32
    else:
        xt_d = [
            nc.dram_tensor("xt_hi", (D, T), FP16, kind="ExternalInput").ap(),
            nc.dram_tensor("xt_lo", (D, T), FP16, kind="ExternalInput").ap(),
        ]
        agt_d = [
            nc.dram_tensor("agt_hi", (D, E), FP16, kind="ExternalInput").ap(),
            nc.dram_tensor("agt_lo", (D, E), FP16, kind="ExternalInput").ap(),
        ]
        DTX = FP16
    NS = len(xt_d)  # number of hi/lo components

    xn = nc.dram_tensor("xn", (T, D), DT2, kind="ExternalInput").ap()
    agb = nc.dram_tensor("agb", (E, 1), FP32, kind="ExternalInput").ap()
    # grouped c-major: wfin[dt, dp, g, c, el] = lm_w[(g*EG+el)*C+c, dt*128+dp]
    wfin = nc.dram_tensor("wfin", (KT, 128, NG, C, EG), FP16, kind="ExternalInput").ap()
    # mask[b*EG+ep, (c, e)] = (e == ep); selects the diagonal e-blocks of the
    # final-stage cross-product matmuls
    mask = nc.dram_tensor("mask", (128, 2, C, EG), FP16, kind="ExternalInput").ap()
    # sel3[b*EG+ep, bp] = (b == bp); partition-sums R pieces per batch
    sel3 = nc.dram_tensor("sel3", (128, BL), FP16, kind="ExternalInput").ap()
    ident = nc.dram_tensor("ident", (128, 128), BF16, kind="ExternalInput").ap()
    preds_o = nc.dram_tensor("preds_o", (BL, C), FP32, kind="ExternalOutput").ap()

    from contextlib import ExitStack
    with tile.TileContext(nc) as tc, ExitStack() as _es:
        if True:
            xt_p = _es.enter_context(tc.tile_pool(name="xt_p", bufs=1))
            agt_p = _es.enter_context(tc.tile_pool(name="agt_p", bufs=1))
            small_p = _es.enter_context(tc.tile_pool(name="small_p", bufs=1))
            st_p = _es.enter_context(tc.tile_pool(name="st_p", bufs=1))
            sn_p = _es.enter_context(tc.tile_pool(name="sn_p", bufs=1))
            xn_p = _es.enter_context(tc.tile_pool(name="xn_p", bufs=1))
            gt_p = _es.enter_context(tc.tile_pool(name="gt_p", bufs=1))
            wfin_p = _es.enter_context(tc.tile_pool(name="wfin_p", bufs=1))
            out_p = _es.enter_context(tc.tile_pool(name="out_p", bufs=1))
            msk_p = _es.enter_context(tc.tile_pool(name="msk_p", bufs=1))
            r_p = _es.enter_context(tc.tile_pool(name="r_p", bufs=1))
            m_p = _es.enter_context(tc.tile_pool(name="m_p", bufs=4))
            ps1 = _es.enter_context(tc.tile_pool(name="ps1", bufs=2, space="PSUM"))
            pst = _es.enter_context(tc.tile_pool(name="pst", bufs=2, space="PSUM"))
            ps2 = _es.enter_context(tc.tile_pool(name="ps2", bufs=2, space="PSUM"))
            psM = _es.enter_context(tc.tile_pool(name="psM", bufs=2, space="PSUM"))
                        # ---- PE warm-up: HAM releases the PE clock gate (1.2 -> 2.4
            # GHz) only after ~3.4us of sustained matmul activity. The first
            # ~12us of the kernel is DMA-bound, which would keep the PE cold
            # into mm1. Fill the wait with dummy matmuls on memset scratch.
            warm_src = small_p.tile([128, W1], FP16, tag="warm_src",
                                    name="warm_src")
            nc.gpsimd.memset(warm_src[:], 0.0)
            warm_w = small_p.tile([128, 128], FP16, tag="warm_w",
                                  name="warm_w")
            nc.gpsimd.memset(warm_w[:], 0.0)
            for wi in range(14):
                wps = ps1.tile([128, W1], FP32, tag="ps1", name=f"warm_ps{wi}")
                nc.tensor.matmul(
                    wps[:], warm_w[:], warm_src[:], start=True, stop=True
                )

            # ---- persistent loads on the SP queue, issued in consumption
            # order (SP descriptor issue is ~0.6us each, so few + ordered).
            agb_sb = []
            for et in range(ET):
                t = small_p.tile([128, 1], FP32, tag=f"agb{et}",
                                 name=f"agb_sb{et}")
                nc.gpsimd.dma_start(t[:], agb[et * 128:(et + 1) * 128, :])
                agb_sb.append(t)
            agt_sb = [
                [
                    agt_p.tile([128, E], DTX, tag=f"agt{s}_{kt}",
                               name=f"agt_sb{s}_{kt}")
                    for kt in range(KT)
                ]
                for s in range(NS)
            ]
            xt_sb = [
                [
                    xt_p.tile([128, T], DTX, tag=f"xt{s}_{kt}",
                              name=f"xt_sb{s}_{kt}")
                    for kt in range(KT)
                ]
                for s in range(NS)
            ]
            for kt in range(KT):
                for s in range(NS):
                    eng = nc.sync if s == 0 else nc.scalar
                    eng.dma_start(
                        agt_sb[s][kt][:], agt_d[s][kt * 128:(kt + 1) * 128, :]
                    )
            for hf in range(2):
                for kt in range(KT):
                    for s in range(NS):
                        eng = nc.sync if s == 0 else nc.scalar
                        eng.dma_start(
                            xt_sb[s][kt][:, hf * (T // 2):(hf + 1) * (T // 2)],
                            xt_d[s][kt * 128:(kt + 1) * 128,
                                    hf * (T // 2):(hf + 1) * (T // 2)],
                        )
            ident_sb = small_p.tile([128, 128], BF16, tag="ident")
            nc.gpsimd.dma_start(ident_sb[:], ident[:, :])

            # ---- mm1: S^T[e,t] = (agt^T @ xt + b) > 0 ----
            # fp16x3: x@w ~= xhi@whi + xhi@wlo + xlo@whi  (lo*lo negligible)
            mm1_passes = [(0, 0)] if mm1 == "fp32" else [(0, 0), (0, 1), (1, 0)]
            st_sb = [
                st_p.tile([128, T], BF16, tag=f"st{et}", name=f"st_sb{et}")
                for et in range(ET)
            ]
            sn_sb = []
            for b in range(BL):
                sn_sb.append(
                    [
                        sn_p.tile([128, E], DT2, tag=f"sn{b}_{ht}",
                                  name=f"sn_sb{b}_{ht}")
                        for ht in range(2)
                    ]
                )
            th_insts = {}
            for nch in range(NCH if "mm1" in phases else 0):
                for et in range(ET):
                    ps = ps1.tile([128, W1], FP32, tag="ps1",
                                  name=f"ps1_{et}_{nch}")
                    n_acc = KT * len(mm1_passes)
                    i_acc = 0
                    for kt in range(KT):
                        for (sx, sw) in mm1_passes:
                            nc.tensor.matmul(
                                ps[:],
                                agt_sb[sw][kt][:, et * 128:(et + 1) * 128],
                                xt_sb[sx][kt][:, nch * W1:(nch + 1) * W1],
                                start=(i_acc == 0),
                                stop=(i_acc == n_acc - 1),
                            )
                            i_acc += 1
                    th_insts[(et, nch)] = nc.vector.tensor_scalar(
                        st_sb[et][:, nch * W1:(nch + 1) * W1],
                        ps[:],
                        agb_sb[et][:],
                        0.0,
                        mybir.AluOpType.add,
                        mybir.AluOpType.is_gt,
                    )
                if "tr" in phases:
                    for b in (2 * nch, 2 * nch + 1):
                        for ht in range(2):
                            w = 128 if ht == 0 else HW - 128
                            for et in range(ET):
                                pt = pst.tile([128, 128], BF16, tag="pst",
                                              name=f"pst_{b}_{ht}_{et}")
                                nc.tensor.transpose(
                                    pt[0:w, :],
                                    st_sb[et][:, b * HW + ht * 128:
                                              b * HW + ht * 128 + w],
                                    ident_sb[:],
                                )
                                if (b + et) % 2 == 0:
                                    nc.vector.tensor_copy(
                                        sn_sb[b][ht][0:w,
                                                     et * 128:(et + 1) * 128],
                                        pt[0:w, :],
                                    )
                                else:
                                    nc.scalar.copy(
                                        sn_sb[b][ht][0:w,
                                                     et * 128:(et + 1) * 128],
                                        pt[0:w, :],
                                    )

            # ---- transpose S^T -> S natural (per batch, h on partitions) ----

            # ---- mm2 + final, dt-pipelined ----
            # grouped layout: gt[dt][dp, g, b, el] = G^T_b[dt*128+dp, g*EG+el]
            gt_sb = [
                gt_p.tile([128, NG, BL, EG], FP16, tag=f"gt{dt}",
                          name=f"gt_sb{dt}")
                for dt in range(KT)
            ]
            # all xn tiles upfront (persistent; 24KB): frees the dt loop to
            # run mm2(dt) immediately followed by the final-stage work for dt,
            # keeping the PE dense (HAM stays at full clock)
            xn_sb = []
            for b in range(BL):
                row = []
                for ht in range(2):
                    w = 128 if ht == 0 else HW - 128
                    t = xn_p.tile([128, D], DT2, tag=f"xn{b}_{ht}",
                                  name=f"xn_{b}_{ht}")
                    dma = nc.scalar.dma_start(
                        t[0:w, :],
                        xn[b * HW + ht * 128: b * HW + ht * 128 + w, :],
                    )
                    th = th_insts.get((0, b // 2))
                    if th is not None:
                        add_dep_helper(dma.ins, th.ins,
                                       reason="stagger xn behind mm1 et0")
                    row.append(t)
                xn_sb.append(row)

            # ---- final stage ----
            # For each (dt, e-group of EG): one fp16 matmul computes ALL cross
            # products M[(b,ep), (c,e)] = sum_dp G[dp,(b,ep)] * W[dp,(c,e)]
            # with a [128,128] stationary (FWL-fast) and 160 moving rows.
            # The diagonal blocks (e == ep) are selected by a constant mask
            # (DVE), reduced over e (DVE, innermost X axis), and the resulting
            # R[(b,ep), c] pieces are partition-summed per batch by a constant
            # selection matmul accumulating into one [BL, C] psum tile.
            wfin_sb = []
            for dt in range(KT):
                t = wfin_p.tile([128, NG, C, EG], FP16, tag=f"wfin{dt}",
                                name=f"wfin_sb{dt}")
                dma = nc.gpsimd.dma_start(t[:], wfin[dt])
                th = th_insts.get((0, 1))
                if th is not None:
                    add_dep_helper(dma.ins, th.ins,
                                   reason="wfin load after mm1 done")
                wfin_sb.append(t)
            mask_sb = msk_p.tile([128, 2, C, EG], FP16, tag="mask",
                                 name="mask_sb")
            nc.gpsimd.dma_start(mask_sb[:], mask[:, :, :, :])
            sel3_sb = msk_p.tile([128, BL], FP16, tag="sel3", name="sel3_sb")
            nc.gpsimd.dma_start(sel3_sb[:], sel3[:, :])

            # bridge the mm1->mm2 handoff (PE would otherwise idle on the
            # sn-copy chain long enough for HAM to re-throttle the clock)
            for wi in range(12):
                wps = ps1.tile([128, W1], FP32, tag="ps1",
                               name=f"warm2_ps{wi}")
                nc.tensor.matmul(
                    wps[:], warm_w[:], warm_src[:], start=True, stop=True
                )

            do_fin = "fin" in phases
            do_mm2 = "mm2" in phases
            # pf accumulates pass3 results over all (dt, g): [BL, (c, e)]
            pf = ps1.tile([BL, C, EG], FP32, tag="ps1", name="psf_t")
            if not do_fin:
                nc.vector.memset(pf[:], 0.0)
            ip = 0
            for dt in range(KT):
                for b in range(BL if do_mm2 else 0):
                    pg = ps2.tile([128, E], FP32, tag="ps2", name=f"ps2_{b}_{dt}")
                    for ht in range(2):
                        w = 128 if ht == 0 else HW - 128
                        nc.tensor.matmul(
                            pg[:],
                            xn_sb[b][ht][0:w, dt * 128:(dt + 1) * 128],
                            sn_sb[b][ht][0:w, :],
                            start=(ht == 0),
                            stop=(ht == 1),
                        )
                    if b % 2 == 0:
                        nc.vector.tensor_copy(gt_sb[dt][:, :, b, :], pg[:])
                    else:
                        nc.scalar.copy(gt_sb[dt][:, :, b, :], pg[:])
                if not do_fin:
                    continue
                for gp in range(NG // 2):
                    pm = psM.tile([128, 2, C, EG], FP32, tag="psM",
                                  name=f"psM_{dt}_{gp}")
                    for h in range(2):
                        g = gp * 2 + h
                        nc.tensor.matmul(
                            pm[:, h, :, :],
                            gt_sb[dt][:, g, :, :],
                            wfin_sb[dt][:, g, :, :],
                            start=True,
                            stop=True,
                        )
                    msb = m_p.tile([128, 2, C, EG], FP16, tag="msb",
                                   name=f"msb_{dt}_{gp}")
                    nc.vector.tensor_tensor(
                        msb[:], pm[:], mask_sb[:], mybir.AluOpType.mult
                    )
                    for h in range(2):
                        nc.tensor.matmul(
                            pf[:],
                            sel3_sb[:],
                            msb[:, h, :, :],
                            start=(ip == 0),
                            stop=(ip == KT * NG - 1),
                        )
                        ip += 1
            # final tiny reduction over e + scale
            red_sb = out_p.tile([BL, C], FP32, tag="red", name="red_sb")
            nc.vector.tensor_reduce(
                red_sb[:], pf[:], mybir.AxisListType.X, mybir.AluOpType.add
            )
            out_sb = out_p.tile([BL, C], FP32, tag="out", name="out_sb")
            nc.vector.tensor_scalar_mul(out_sb[:], red_sb[:], 1.0 / (HW * E))
            nc.sync.dma_start(preds_o[:, :], out_sb[:])

    return nc


_program_cache = {}

# Chosen production configuration.
CONFIG = {"mm1": "fp16x3", "mm2": "fp16", "fin": "fp16"}


def _get_program(**kw):
    cfg = dict(CONFIG)
    cfg.update(kw)
    key = tuple(sorted(cfg.items()))
    if key not in _program_cache:
        _program_cache[key] = build_program(**cfg)
    return _program_cache[key]


def _fp16_split(a):
    hi = a.astype(np.float16)
    lo = (a - hi.astype(np.float32)).astype(np.float16)
    return hi, lo


def make_in_maps(x, ag_w, ag_b, lm_w, cfg):
    import ml_dtypes

    x = np.ascontiguousarray(np.asarray(x, dtype=np.float32))
    ag_w = np.asarray(ag_w, dtype=np.float32)
    ag_b = np.asarray(ag_b, dtype=np.float32)
    lm_w = np.asarray(lm_w, dtype=np.float32)

    agb = np.ascontiguousarray(ag_b.reshape(E, 1))
    # c-major, e-innermost: wfin[dt,dp,c,e] = lm_w[e*C+c, dt*128+dp]
    # (fp16; the 1/(HW*E) scale is applied on-chip at the end)
    wfin = np.ascontiguousarray(
        lm_w.T.reshape(KT, 128, NG, EG, C)
        .transpose(0, 1, 2, 4, 3)
        .astype(np.float16)
    )
    ident = np.eye(128, dtype=ml_dtypes.bfloat16)
    ep = np.arange(128) % EG
    e_in = np.arange(EG)
    mask1 = (ep[:, None, None] == e_in[None, None, :]) * np.ones(
        (128, C, EG), dtype=np.float16
    )
    mask = np.ascontiguousarray(
        np.broadcast_to(mask1[:, None, :, :], (128, 2, C, EG)).astype(np.float16)
    )
    bidx = np.arange(128) // EG
    sel3 = (bidx[:, None] == np.arange(BL)[None, :]).astype(np.float16)

    agt = np.ascontiguousarray(ag_w.T)
    common = {"agb": agb, "wfin": wfin, "ident": ident, "mask": mask,
              "sel3": sel3}
    if cfg["mm1"] == "fp32":
        common["agt"] = agt
    else:
        common["agt_hi"], common["agt_lo"] = _fp16_split(agt)

    in_maps = []
    for i in range(NCORES):
        xs = x[i * BL:(i + 1) * BL].reshape(T, D)
        m = dict(common)
        xn_dt = np.float16 if cfg.get("mm2", "fp16") == "fp16" else np.float32
        m["xn"] = np.ascontiguousarray(xs.astype(xn_dt))
        xt_i = np.ascontiguousarray(xs.T)
        if cfg["mm1"] == "fp32":
            m["xt"] = xt_i
        else:
            m["xt_hi"], m["xt_lo"] = _fp16_split(xt_i)
        in_maps.append(m)
    return in_maps


def kernel(x, ag_w, ag_b, lm_w):
    cfg = dict(CONFIG)
    in_maps = make_in_maps(x, ag_w, ag_b, lm_w, cfg)
    nc = _get_program()
    res = run_bass_kernel_spmd(nc, in_maps, core_ids=list(range(NCORES)))
    preds = np.concatenate(
        [res.results[i]["preds_o"] for i in range(NCORES)], axis=0
    )
    return np.ascontiguousarray(preds.astype(np.float32))



# revision 5
# speedup vs baseline: 1.2988x; 1.2988x over previous
"""Trainium2 Bass kernel for nn_ConvexMLPBlock.

Reference computation (B=64, HW=196, D=768, E=256, C=10):
    S[b,h,e]  = (x[b,h,:] @ ag_w[e,:] + ag_b[e]) > 0          (sign patterns)
    z[b,h,p]  = x[b,h,:] @ lm_w[p,:]        (p = e*C + c)
    preds[b,c] = sum_{h,e} S[b,h,e] * z[b,h,e,c] / (HW*E)

Restructured to avoid materializing z (49 GFLOP -> ~10 GFLOP):
    G_b[e,d]   = sum_h S[b,h,e] * x[b,h,d]                    (per-batch masked moment)
    preds[b,c] = (1/(HW*E)) * sum_{e,d} G_b[e,d] * W[e,c,d]   (W = lm_w.reshape(E,C,D))

Sharding: data-parallel over B across the 8 NeuronCores (8 batches/core);
host concatenates the per-core (8,10) outputs.

Per-core pipeline (v2 — single-pass fp16 mm1 in S-natural layout):
    mm1: S[t,e] directly (stationary = x^T d-chunks, moving = ag^T [d,256]),
         ONE fp16 pass (measured rel err ~1.0e-2 < 2e-2 gate; fp16 products
         are exact in the PE, error comes only from operand rounding).
         No PE transposes needed — S lands with h on partitions.
    threshold: DVE tensor_tensor is_gt vs a broadcast (-ag_b) tile.
    mm2: G^T_b[d,e] contraction over h (stationary = x natural d-slices,
         moving = S), 2 h-tiles per batch, fp16.
    final: per e-group g (16 e's): 6 accumulating cross-product matmuls
           (stationary G^T[d,(b,e)], moving W[d,(c,e)]) into one PSUM tile;
           diagonal e-blocks selected by a constant mask (DVE), then one
           sel3 matmul partition-sums per batch, accumulating over groups
           into a single [8,(c,e)] PSUM tile; final e-reduce + scale on DVE.
"""

import numpy as np

import concourse.bass as bass
import concourse.mybir as mybir
import concourse.tile as tile
from concourse.tile import add_dep_helper
from concourse.bass_utils import run_bass_kernel_spmd

# Problem constants (hardcoded per contract).
B = 64
HW = 196
D = 768
E = 256
C = 10
NCORES = 8
BL = B // NCORES          # local batches per core = 8
T = BL * HW               # local tokens = 1568
KT = D // 128             # 6 d-tiles
EG = 16                   # e's per final-stage group
NG = E // EG              # 16 groups

FP32 = mybir.dt.float32
F32R = mybir.dt.float32r
BF16 = mybir.dt.bfloat16
FP16 = mybir.dt.float16


def _patched_drain_and_barrier(self, tick_clock, wait_clock):
    """This toolchain's walrus rejects >1 sync-wait on CTRL-class (Drain)
    instructions. Split the tail drain's global-clock waits across multiple
    single-wait drains. Semantics preserved: SP observes every DMA-queue
    semaphore before the all-engine barrier."""
    drain_inst = self.nc.sync.drain()
    wait_clock.add_sem_waits(
        drain_inst.ins, tile.ScopedClock({None: tick_clock.global_clock})
    )
    si = drain_inst.ins.sync_info
    if si is not None and si.on_wait is not None and len(si.on_wait) > 1:
        waits = list(si.on_wait)
        drain_inst.ins.sync_info = mybir.SyncInfo(
            on_wait=[waits[0]], on_update=list(si.on_update or [])
        )
        for w in waits[1:]:
            extra = self.nc.sync.drain()
            extra.ins.sync_info = mybir.SyncInfo(on_wait=[w], on_update=[])

    self.nc.all_engine_barrier()
    assert self.sems is not None
    popped = self.nc._tile_sem_poison_stack.pop()
    assert popped is self._sem_poison
    self.nc.clear_and_free_semaphores(list(self.sems.allocated().values()))
    self.nc.all_engine_barrier()


tile.TileContext._drain_and_barrier = _patched_drain_and_barrier


def _split_multiwait_json(bj: bytes) -> bytes:
    """Walrus in this toolchain accepts at most one sync-wait per instruction.
    For any instruction with N>1 waits, hoist N-1 waits onto same-engine NoOps
    inserted immediately before it. Engines execute program-order, so for
    compute instructions this is semantically identical; for DMAs it
    conservatively blocks the issuing engine instead of the queue."""
    import json

    m = json.loads(bj)
    changed = False
    for fn in m["functions"]:
        for bb in fn["blocks"]:
            new_insts = []
            for inst in bb["instructions"]:
                si = inst.get("sync_info")
                ow = (si or {}).get("on_wait") or []
                if len(ow) > 1:
                    for j, w in enumerate(ow[:-1]):
                        new_insts.append(
                            {
                                "name": f"{inst['name']}__w{j}",
                                "opcode": "NoOp",
                                "engine": inst["engine"],
                                "ins": [],
                                "outs": [],
                                "sync_info": {"on_update": [], "on_wait": [w]},
                            }
                        )
                    si["on_wait"] = [ow[-1]]
                    changed = True
                new_insts.append(inst)
            bb["instructions"] = new_insts
    if not changed:
        return bj
    return json.dumps(m).encode()


_orig_to_json_bytes = bass.Bass.to_json_bytes


def _patched_to_json_bytes(self, *a, **k):
    return _split_multiwait_json(_orig_to_json_bytes(self, *a, **k))


bass.Bass.to_json_bytes = _patched_to_json_bytes


# (batch, half) chunks: per batch a 128-row and a 68-row h-chunk.
CHUNKS = []
for _b in range(BL):
    CHUNKS.append((_b, 0, _b * HW, 128))
    CHUNKS.append((_b, 1, _b * HW + 128, HW - 128))


def build_program(phases=("mm1", "mm2", "fin")):
    nc = bass.Bass()

    # xt[dp, kt, t] = x_core[t, kt*128+dp]       (fp16, mm1 stationary)
    xt_d = nc.dram_tensor("xt", (128, KT, T), FP16, kind="ExternalInput").ap()
    # agt[dp, kt, e] = ag_w[e, kt*128+dp]        (fp16, mm1 moving)
    agt_d = nc.dram_tensor("agt", (128, KT, E), FP16, kind="ExternalInput").ap()
    # negb[p, e] = -ag_b[e]                      (fp32, threshold)
    negb_d = nc.dram_tensor("negb", (128, E), FP32, kind="ExternalInput").ap()
    # xn[t, d] = x_core[t, d]                    (fp16, mm2 stationary)
    xn_d = nc.dram_tensor("xn", (T, D), FP16, kind="ExternalInput").ap()
    # wfin[dp, kt, g, c, el] = lm_w[(g*EG+el)*C+c, kt*128+dp]   (fp16)
    wfin_d = nc.dram_tensor("wfin", (128, KT, NG, C, EG), FP16,
                            kind="ExternalInput").ap()
    # mask[b*EG+ep, c, el] = (ep == el)
    mask_d = nc.dram_tensor("mask", (128, C, EG), FP16, kind="ExternalInput").ap()
    # sel3[b*EG+ep, bp] = (b == bp)
    sel3_d = nc.dram_tensor("sel3", (128, BL), FP16, kind="ExternalInput").ap()
    preds_o = nc.dram_tensor("preds_o", (BL, C), FP32, kind="ExternalOutput").ap()

    from contextlib import ExitStack
    with tile.TileContext(nc) as tc, ExitStack() as _es:
        xt_p = _es.enter_context(tc.tile_pool(name="xt_p", bufs=1))
        agt_p = _es.enter_context(tc.tile_pool(name="agt_p", bufs=1))
        small_p = _es.enter_context(tc.tile_pool(name="small_p", bufs=1))
        sn_p = _es.enter_context(tc.tile_pool(name="sn_p", bufs=1))
        xn_p = _es.enter_context(tc.tile_pool(name="xn_p", bufs=1))
        gt_p = _es.enter_context(tc.tile_pool(name="gt_p", bufs=1))
        wfin_p = _es.enter_context(tc.tile_pool(name="wfin_p", bufs=1))
        out_p = _es.enter_context(tc.tile_pool(name="out_p", bufs=1))
        m_p = _es.enter_context(tc.tile_pool(name="m_p", bufs=4))
        ps1 = _es.enter_context(tc.tile_pool(name="ps1", bufs=2, space="PSUM"))
        ps2 = _es.enter_context(tc.tile_pool(name="ps2", bufs=2, space="PSUM"))
        psM = _es.enter_context(tc.tile_pool(name="psM", bufs=2, space="PSUM"))
        psF = _es.enter_context(tc.tile_pool(name="psF", bufs=1, space="PSUM"))

        # ---- PE warm-up: HAM releases the PE clock gate (1.2 -> 2.4 GHz)
        # only after ~3.4us of sustained matmul activity. The first few us
        # are DMA-bound; fill them with dummy matmuls on memset scratch.
        warm_src = small_p.tile([128, E], FP16, tag="warm_src",
                                name="warm_src")
        nc.gpsimd.memset(warm_src[:], 0.0)
        warm_w = small_p.tile([128, 128], FP16, tag="warm_w", name="warm_w")
        nc.gpsimd.memset(warm_w[:], 0.0)
        for wi in range(14):
            wps = ps1.tile([128, E], FP32, tag="ps1", name=f"warm_ps{wi}")
            nc.tensor.matmul(
                wps[:], warm_w[:], warm_src[:], start=True, stop=True
            )

        # ---- persistent loads; sync (SP ring) carries the mm1-critical
        # tensors, scalar (ACT ring) carries xn, gpsimd (SWDGE) wfin.
        agt_sb = agt_p.tile([128, KT, E], FP16, tag="agt", name="agt_sb")
        nc.sync.dma_start(agt_sb[:], agt_d[:, :, :])
        negb_sb = small_p.tile([128, E], FP32, tag="negb", name="negb_sb")
        nc.sync.dma_start(negb_sb[:], negb_d[:, :])
        xt_sb = xt_p.tile([128, KT, T], FP16, tag="xt", name="xt_sb")
        NXP = 4  # xt DMA pieces
        xt_dmas = []
        for pi in range(NXP):
            t0, t1 = pi * (T // NXP), (pi + 1) * (T // NXP)
            xt_dmas.append(nc.sync.dma_start(
                xt_sb[:, :, t0:t1], xt_d[:, :, t0:t1]
            ))

        mask_sb = small_p.tile([128, C, EG], FP16, tag="mask", name="mask_sb")
        nc.scalar.dma_start(mask_sb[:], mask_d[:, :, :])
        sel3_sb = small_p.tile([128, BL], FP16, tag="sel3", name="sel3_sb")
        nc.scalar.dma_start(sel3_sb[:], sel3_d[:, :])

        # ---- mm1: S[t,e] = (x @ ag_w^T - (-b)) > 0, single fp16 pass ----
        sn_sb = [
            sn_p.tile([128, E], FP16, tag=f"sn{ch}", name=f"sn_sb{ch}")
            for ch in range(len(CHUNKS))
        ]
        th_insts = {}
        for ch, (b, ht, t0, w) in enumerate(CHUNKS if "mm1" in phases else []):
            ps = ps1.tile([128, E], FP32, tag="ps1", name=f"ps1_{ch}")
            for kt in range(KT):
                nc.tensor.matmul(
                    ps[0:w, :],
                    xt_sb[:, kt, t0:t0 + w],
                    agt_sb[:, kt, :],
                    start=(kt == 0),
                    stop=(kt == KT - 1),
                )
            th_insts[ch] = nc.vector.tensor_tensor(
                sn_sb[ch][0:w, :], ps[0:w, :], negb_sb[0:w, :],
                mybir.AluOpType.is_gt,
            )

        # xn tiles: stream on the ACT ring, staggered behind mm1 progress so
        # the early HBM bandwidth goes to xt.
        xn_sb = []
        for ch, (b, ht, t0, w) in enumerate(CHUNKS):
            t_tile = xn_p.tile([128, D], FP16, tag=f"xn{ch}", name=f"xn_{ch}")
            dma = nc.scalar.dma_start(t_tile[0:w, :], xn_d[t0:t0 + w, :])
            th = th_insts.get(max(0, ch - 4))
            if th is not None and ch >= 4:
                add_dep_helper(dma.ins, th.ins,
                               reason="stagger xn behind mm1")
            xn_sb.append(t_tile)

        # wfin: SWDGE ring, start after mm1 is underway.
        wfin_sb = wfin_p.tile([128, KT, NG, C, EG], FP16, tag="wfin",
                              name="wfin_sb")
        for gh in range(2):
            g0, g1 = gh * (NG // 2), (gh + 1) * (NG // 2)
            dma = nc.gpsimd.dma_start(
                wfin_sb[:, :, g0:g1, :, :], wfin_d[:, :, g0:g1, :, :]
            )
            th = th_insts.get(2 + gh)
            if th is not None:
                add_dep_helper(dma.ins, th.ins,
                               reason="wfin load after mm1 start")

        # ---- mm2: G^T_b[d, e] = sum_h x[h,d] S[h,e] ----
        # grouped layout: gt[dt][dp, g, b, el] = G^T_b[dt*128+dp, g*EG+el]
        gt_sb = [
            gt_p.tile([128, NG, BL, EG], FP16, tag=f"gt{dt}",
                      name=f"gt_sb{dt}")
            for dt in range(KT)
        ]
        for b in range(BL if "mm2" in phases else 0):
            for dt in range(KT):
                pg = ps2.tile([128, E], FP32, tag="ps2", name=f"ps2_{b}_{dt}")
                for ht in range(2):
                    ch = 2 * b + ht
                    w = CHUNKS[ch][3]
                    nc.tensor.matmul(
                        pg[:],
                        xn_sb[ch][0:w, dt * 128:(dt + 1) * 128],
                        sn_sb[ch][0:w, :],
                        start=(ht == 0),
                        stop=(ht == 1),
                    )
                if (b + dt) % 2 == 0:
                    nc.vector.tensor_copy(gt_sb[dt][:, :, b, :], pg[:])
                else:
                    nc.scalar.copy(gt_sb[dt][:, :, b, :], pg[:])

        # ---- final: cross-product matmuls + mask select + sel3 reduce ----
        do_fin = "fin" in phases
        pf = psF.tile([BL, C, EG], FP32, tag="psf", name="psf_t")
        if not do_fin:
            nc.vector.memset(pf[:], 0.0)
        for g in range(NG if do_fin else 0):
            pm = psM.tile([128, C, EG], FP32, tag="psM", name=f"psM_{g}")
            for dt in range(KT):
                nc.tensor.matmul(
                    pm[:],
                    gt_sb[dt][:, g, :, :],
                    wfin_sb[:, dt, g, :, :],
                    start=(dt == 0),
                    stop=(dt == KT - 1),
                )
            msb = m_p.tile([128, C, EG], FP16, tag="msb", name=f"msb_{g}")
            nc.vector.tensor_tensor(
                msb[:], pm[:], mask_sb[:], mybir.AluOpType.mult
            )
            nc.tensor.matmul(
                pf[:], sel3_sb[:], msb[:],
                start=(g == 0), stop=(g == NG - 1),
            )

        # final tiny reduction over e + scale
        red_sb = out_p.tile([BL, C], FP32, tag="red", name="red_sb")
        nc.vector.tensor_reduce(
            red_sb[:], pf[:], mybir.AxisListType.X, mybir.AluOpType.add
        )
        out_sb = out_p.tile([BL, C], FP32, tag="out", name="out_sb")
        nc.vector.tensor_scalar_mul(out_sb[:], red_sb[:], 1.0 / (HW * E))
        nc.sync.dma_start(preds_o[:, :], out_sb[:])

    return nc


_program_cache = {}

CONFIG = {}


def _get_program(**kw):
    key = tuple(sorted(kw.items()))
    if key not in _program_cache:
        _program_cache[key] = build_program(**kw)
    return _program_cache[key]


def make_in_maps(x, ag_w, ag_b, lm_w, cfg=None):
    x = np.ascontiguousarray(np.asarray(x, dtype=np.float32))
    ag_w = np.asarray(ag_w, dtype=np.float32)
    ag_b = np.asarray(ag_b, dtype=np.float32)
    lm_w = np.asarray(lm_w, dtype=np.float32)

    agt = np.ascontiguousarray(
        ag_w.T.reshape(KT, 128, E).transpose(1, 0, 2).astype(np.float16)
    )
    negb = np.ascontiguousarray(
        np.broadcast_to(-ag_b[None, :], (128, E)).astype(np.float32)
    )
    # wfin[dp, kt, g, c, el] = lm_w[(g*EG+el)*C+c, kt*128+dp]
    wfin = np.ascontiguousarray(
        lm_w.T.reshape(KT, 128, NG, EG, C)
        .transpose(1, 0, 2, 4, 3)
        .astype(np.float16)
    )
    ep = np.arange(128) % EG
    mask = np.ascontiguousarray(
        (ep[:, None, None] == np.arange(EG)[None, None, :])
        * np.ones((128, C, EG), dtype=np.float16)
    )
    bidx = np.arange(128) // EG
    sel3 = (bidx[:, None] == np.arange(BL)[None, :]).astype(np.float16)

    common = {"agt": agt, "negb": negb, "wfin": wfin, "mask": mask,
              "sel3": sel3}
    in_maps = []
    for i in range(NCORES):
        xs = x[i * BL:(i + 1) * BL].reshape(T, D)
        m = dict(common)
        m["xn"] = np.ascontiguousarray(xs.astype(np.float16))
        m["xt"] = np.ascontiguousarray(
            xs.T.reshape(KT, 128, T).transpose(1, 0, 2).astype(np.float16)
        )
        in_maps.append(m)
    return in_maps


def kernel(x, ag_w, ag_b, lm_w):
    in_maps = make_in_maps(x, ag_w, ag_b, lm_w)
    nc = _get_program()
    res = run_bass_kernel_spmd(nc, in_maps, core_ids=list(range(NCORES)))
    preds = np.concatenate(
        [res.results[i]["preds_o"] for i in range(NCORES)], axis=0
    )
    return np.ascontiguousarray(preds.astype(np.float32))


# revision 6
# speedup vs baseline: 1.6568x; 1.2756x over previous
"""Trainium2 Bass kernel for nn_ConvexMLPBlock.

Reference computation (B=64, HW=196, D=768, E=256, C=10):
    S[b,h,e]  = (x[b,h,:] @ ag_w[e,:] + ag_b[e]) > 0          (sign patterns)
    z[b,h,p]  = x[b,h,:] @ lm_w[p,:]        (p = e*C + c)
    preds[b,c] = sum_{h,e} S[b,h,e] * z[b,h,e,c] / (HW*E)

Restructured to avoid materializing z (49 GFLOP -> ~10 GFLOP):
    G_b[e,d]   = sum_h S[b,h,e] * x[b,h,d]                    (per-batch masked moment)
    preds[b,c] = (1/(HW*E)) * sum_{e,d} G_b[e,d] * W[e,c,d]   (W = lm_w.reshape(E,C,D))

Sharding: data-parallel over B across the 8 NeuronCores (8 batches/core);
host concatenates the per-core (8,10) outputs.

Per-core pipeline (v3):
    mm1: S[t,e] directly (stationary = x^T d-chunks, moving = ag^T [d,256]),
         ONE fp16 pass (rel err ~1.0e-2 < 2e-2 gate; fp16 products are exact
         in the PE, error comes only from operand rounding). No transposes.
    threshold: DVE tensor_tensor is_gt vs a broadcast (-ag_b) tile.
    mm2: G^T_b[d,e] contraction over h (stationary = x natural d-slices,
         moving = S), 2 h-tiles per batch, fp16.
    final: the e-diagonal selection mask is group-independent, so ALL 96
           cross-product matmuls (per d-tile and e-group: stationary
           G^T[d,(b,e)], moving W[d,(c,e)]) accumulate into a single
           [128,(c,e)] PSUM tile; then one mask-mult (DVE), one sel3
           partition-sum matmul, one e-reduce + scale.
    All DMAs ride the two HWDGE rings (SP + ACT) with per-partition
    contiguous layouts; SWDGE (gpsimd) is avoided entirely.
"""

import numpy as np

import concourse.bass as bass
import concourse.mybir as mybir
import concourse.tile as tile
from concourse.tile import add_dep_helper
from concourse.bass_utils import run_bass_kernel_spmd

# Problem constants (hardcoded per contract).
B = 64
HW = 196
D = 768
E = 256
C = 10
NCORES = 8
BL = B // NCORES          # local batches per core = 8
T = BL * HW               # local tokens = 1568
KT = D // 128             # 6 d-tiles
EG = 16                   # e's per final-stage group
NG = E // EG              # 16 groups

FP32 = mybir.dt.float32
BF16 = mybir.dt.bfloat16
FP16 = mybir.dt.float16


def _patched_drain_and_barrier(self, tick_clock, wait_clock):
    """This toolchain's walrus rejects >1 sync-wait on CTRL-class (Drain)
    instructions. Split the tail drain's global-clock waits across multiple
    single-wait drains. Semantics preserved: SP observes every DMA-queue
    semaphore before the all-engine barrier."""
    drain_inst = self.nc.sync.drain()
    wait_clock.add_sem_waits(
        drain_inst.ins, tile.ScopedClock({None: tick_clock.global_clock})
    )
    si = drain_inst.ins.sync_info
    if si is not None and si.on_wait is not None and len(si.on_wait) > 1:
        waits = list(si.on_wait)
        drain_inst.ins.sync_info = mybir.SyncInfo(
            on_wait=[waits[0]], on_update=list(si.on_update or [])
        )
        for w in waits[1:]:
            extra = self.nc.sync.drain()
            extra.ins.sync_info = mybir.SyncInfo(on_wait=[w], on_update=[])

    self.nc.all_engine_barrier()
    assert self.sems is not None
    popped = self.nc._tile_sem_poison_stack.pop()
    assert popped is self._sem_poison
    self.nc.clear_and_free_semaphores(list(self.sems.allocated().values()))
    self.nc.all_engine_barrier()


tile.TileContext._drain_and_barrier = _patched_drain_and_barrier


def _split_multiwait_json(bj: bytes) -> bytes:
    """Walrus in this toolchain accepts at most one sync-wait per instruction.
    For any instruction with N>1 waits, hoist N-1 waits onto same-engine NoOps
    inserted immediately before it. Engines execute program-order, so for
    compute instructions this is semantically identical; for DMAs it
    conservatively blocks the issuing engine instead of the queue."""
    import json

    m = json.loads(bj)
    changed = False
    for fn in m["functions"]:
        for bb in fn["blocks"]:
            new_insts = []
            for inst in bb["instructions"]:
                si = inst.get("sync_info")
                ow = (si or {}).get("on_wait") or []
                if len(ow) > 1:
                    for j, w in enumerate(ow[:-1]):
                        new_insts.append(
                            {
                                "name": f"{inst['name']}__w{j}",
                                "opcode": "NoOp",
                                "engine": inst["engine"],
                                "ins": [],
                                "outs": [],
                                "sync_info": {"on_update": [], "on_wait": [w]},
                            }
                        )
                    si["on_wait"] = [ow[-1]]
                    changed = True
                new_insts.append(inst)
            bb["instructions"] = new_insts
    if not changed:
        return bj
    return json.dumps(m).encode()


_orig_to_json_bytes = bass.Bass.to_json_bytes


def _patched_to_json_bytes(self, *a, **k):
    return _split_multiwait_json(_orig_to_json_bytes(self, *a, **k))


bass.Bass.to_json_bytes = _patched_to_json_bytes


# (batch, half) chunks: per batch a 128-row and a 68-row h-chunk.
CHUNKS = []
for _b in range(BL):
    CHUNKS.append((_b, 0, 0, 128))
    CHUNKS.append((_b, 1, 128, HW - 128))


def build_program(phases=("mm1", "mm2", "fin")):
    nc = bass.Bass()

    # xt[dp, b, kt, h] = x_core[b*HW+h, kt*128+dp]   (fp16, mm1 stationary)
    xt_d = nc.dram_tensor("xt", (128, BL, KT, HW), FP16,
                          kind="ExternalInput").ap()
    # agt[dp, kt, e] = ag_w[e, kt*128+dp]            (fp16, mm1 moving)
    agt_d = nc.dram_tensor("agt", (128, KT, E), FP16, kind="ExternalInput").ap()
    # negb[p, e] = -ag_b[e]                          (fp32, threshold)
    negb_d = nc.dram_tensor("negb", (128, E), FP32, kind="ExternalInput").ap()
    # xn[t, d] = x_core[t, d]                        (fp16, mm2 stationary)
    xn_d = nc.dram_tensor("xn", (T, D), FP16, kind="ExternalInput").ap()
    # wfin[dp, gh, kt, gl, c, el] = lm_w[((gh*8+gl)*EG+el)*C+c, kt*128+dp]
    wfin_d = nc.dram_tensor("wfin", (128, 2, KT, NG // 2, C, EG), FP16,
                            kind="ExternalInput").ap()
    # mask[b*EG+ep, c, el] = (ep == el)
    mask_d = nc.dram_tensor("mask", (128, C, EG), FP16,
                            kind="ExternalInput").ap()
    # sel3[b*EG+ep, bp] = (b == bp)
    sel3_d = nc.dram_tensor("sel3", (128, BL), FP16, kind="ExternalInput").ap()
    preds_o = nc.dram_tensor("preds_o", (BL, C), FP32, kind="ExternalOutput").ap()

    from contextlib import ExitStack
    with tile.TileContext(nc) as tc, ExitStack() as _es:
        xt_p = _es.enter_context(tc.tile_pool(name="xt_p", bufs=1))
        agt_p = _es.enter_context(tc.tile_pool(name="agt_p", bufs=1))
        small_p = _es.enter_context(tc.tile_pool(name="small_p", bufs=1))
        sn_p = _es.enter_context(tc.tile_pool(name="sn_p", bufs=1))
        xn_p = _es.enter_context(tc.tile_pool(name="xn_p", bufs=1))
        gt_p = _es.enter_context(tc.tile_pool(name="gt_p", bufs=1))
        wfin_p = _es.enter_context(tc.tile_pool(name="wfin_p", bufs=1))
        out_p = _es.enter_context(tc.tile_pool(name="out_p", bufs=1))
        ps1 = _es.enter_context(tc.tile_pool(name="ps1", bufs=2, space="PSUM"))
        ps2 = _es.enter_context(tc.tile_pool(name="ps2", bufs=3, space="PSUM"))
        psM = _es.enter_context(tc.tile_pool(name="psM", bufs=1, space="PSUM"))
        psF = _es.enter_context(tc.tile_pool(name="psF", bufs=1, space="PSUM"))

        # ---- PE warm-up: HAM releases the PE clock gate (1.2 -> 2.4 GHz)
        # only after ~3.4us of sustained matmul activity; the first few us
        # are DMA-bound. Memsets ride DVE so the warm matmuls start at ~0.
        warm_src = small_p.tile([128, E], FP16, tag="warm_src",
                                name="warm_src")
        nc.vector.memset(warm_src[:], 0.0)
        warm_w = small_p.tile([128, 128], FP16, tag="warm_w", name="warm_w")
        nc.vector.memset(warm_w[:], 0.0)
        for wi in range(16):
            wps = ps1.tile([128, E], FP32, tag="ps1", name=f"warm_ps{wi}")
            nc.tensor.matmul(
                wps[:], warm_w[:], warm_src[:], start=True, stop=True
            )
        # Pre-load the ACT op table (~1.3us, one-time) during the DMA phase
        # so the first real nc.scalar.copy doesn't stall the gt pipeline.
        act_warm = small_p.tile([128, 8], FP16, tag="act_warm",
                                name="act_warm")
        nc.scalar.copy(act_warm[:], warm_w[:, 0:8])

        # ---- persistent loads on the two HWDGE rings, consumption order ----
        agt_sb = agt_p.tile([128, KT, E], FP16, tag="agt", name="agt_sb")
        nc.sync.dma_start(agt_sb[:], agt_d[:, :, :])
        negb_sb = small_p.tile([128, E], FP32, tag="negb", name="negb_sb")
        nc.sync.dma_start(negb_sb[:], negb_d[:, :])
        mask_sb = small_p.tile([128, C, EG], FP16, tag="mask", name="mask_sb")
        nc.scalar.dma_start(mask_sb[:], mask_d[:, :, :])
        sel3_sb = small_p.tile([128, BL], FP16, tag="sel3", name="sel3_sb")
        nc.scalar.dma_start(sel3_sb[:], sel3_d[:, :])

        xt_sb = xt_p.tile([128, BL, KT, HW], FP16, tag="xt", name="xt_sb")
        for b in range(BL):
            nc.sync.dma_start(xt_sb[:, b, :, :], xt_d[:, b, :, :])

        # ---- mm1: S[t,e] = (x @ ag_w^T > -b), single fp16 pass ----
        sn_sb = [
            sn_p.tile([128, E], FP16, tag=f"sn{ch}", name=f"sn_sb{ch}")
            for ch in range(len(CHUNKS))
        ]
        th_insts = {}
        for ch, (b, ht, h0, w) in enumerate(CHUNKS if "mm1" in phases else []):
            ps = ps1.tile([128, E], FP32, tag="ps1", name=f"ps1_{ch}")
            for kt in range(KT):
                nc.tensor.matmul(
                    ps[0:w, :],
                    xt_sb[:, b, kt, h0:h0 + w],
                    agt_sb[:, kt, :],
                    start=(kt == 0),
                    stop=(kt == KT - 1),
                )
            th_insts[ch] = nc.vector.tensor_tensor(
                sn_sb[ch][0:w, :], ps[0:w, :], negb_sb[0:w, :],
                mybir.AluOpType.is_gt,
            )

        # xn tiles on the ACT ring, staggered behind mm1 progress so the
        # early HBM bandwidth goes to xt.
        xn_sb = []
        for ch, (b, ht, h0, w) in enumerate(CHUNKS):
            t_tile = xn_p.tile([128, D], FP16, tag=f"xn{ch}", name=f"xn_{ch}")
            t0 = b * HW + h0
            dma = nc.scalar.dma_start(t_tile[0:w, :], xn_d[t0:t0 + w, :])
            th = th_insts.get(ch - 6)
            if th is not None:
                add_dep_helper(dma.ins, th.ins,
                               reason="stagger xn behind mm1")
            xn_sb.append(t_tile)

        # wfin halves: SP ring (first half, needed at final start) and ACT
        # ring (second half), both after mm1 is underway.
        wfin_sb = wfin_p.tile([128, 2, KT, NG // 2, C, EG], FP16, tag="wfin",
                              name="wfin_sb")
        for gh, eng, chd in ((0, nc.sync, 5), (1, nc.scalar, 9)):
            dma = eng.dma_start(wfin_sb[:, gh, :, :, :, :],
                                wfin_d[:, gh, :, :, :, :])
            th = th_insts.get(chd)
            if th is not None:
                add_dep_helper(dma.ins, th.ins,
                               reason="wfin load after mm1 underway")

        # ---- mm2: G^T_b[d, e] = sum_h x[h,d] S[h,e] ----
        # grouped layout: gt[dt][dp, g, b, el] = G^T_b[dt*128+dp, g*EG+el]
        gt_sb = [
            gt_p.tile([128, NG, BL, EG], FP16, tag=f"gt{dt}",
                      name=f"gt_sb{dt}")
            for dt in range(KT)
        ]
        for b in range(BL if "mm2" in phases else 0):
            for dt in range(KT):
                pg = ps2.tile([128, E], FP32, tag="ps2", name=f"ps2_{b}_{dt}")
                for ht in range(2):
                    ch = 2 * b + ht
                    w = CHUNKS[ch][3]
                    nc.tensor.matmul(
                        pg[:],
                        xn_sb[ch][0:w, dt * 128:(dt + 1) * 128],
                        sn_sb[ch][0:w, :],
                        start=(ht == 0),
                        stop=(ht == 1),
                    )
                if (b + dt) % 2 == 0:
                    nc.vector.tensor_copy(gt_sb[dt][:, :, b, :], pg[:])
                else:
                    nc.scalar.copy(gt_sb[dt][:, :, b, :], pg[:])

        # ---- final ----
        # mask (e-diagonal selection) is identical for every group, so it
        # commutes with the group sum: ALL cross-product matmuls accumulate
        # into one PSUM tile, masked once at the end.
        do_fin = "fin" in phases
        pm = psM.tile([128, C, EG], FP32, tag="psM", name="psM_t")
        if not do_fin:
            nc.vector.memset(pm[:], 0.0)
        nmm = KT * NG
        im = 0
        for g in range(NG if do_fin else 0):
            gh, gl = g // (NG // 2), g % (NG // 2)
            for dt in range(KT):
                nc.tensor.matmul(
                    pm[:],
                    gt_sb[dt][:, g, :, :],
                    wfin_sb[:, gh, dt, gl, :, :],
                    start=(im == 0),
                    stop=(im == nmm - 1),
                )
                im += 1
        msb = out_p.tile([128, C, EG], FP16, tag="msb", name="msb_t")
        nc.vector.tensor_tensor(
            msb[:], pm[:], mask_sb[:], mybir.AluOpType.mult
        )
        pf = psF.tile([BL, C, EG], FP32, tag="psf", name="psf_t")
        nc.tensor.matmul(pf[:], sel3_sb[:], msb[:], start=True, stop=True)

        # final tiny reduction over e + scale
        red_sb = out_p.tile([BL, C], FP32, tag="red", name="red_sb")
        nc.vector.tensor_reduce(
            red_sb[:], pf[:], mybir.AxisListType.X, mybir.AluOpType.add
        )
        out_sb = out_p.tile([BL, C], FP32, tag="out", name="out_sb")
        nc.vector.tensor_scalar_mul(out_sb[:], red_sb[:], 1.0 / (HW * E))
        nc.sync.dma_start(preds_o[:, :], out_sb[:])

    return nc


_program_cache = {}

CONFIG = {}


def _get_program(**kw):
    key = tuple(sorted(kw.items()))
    if key not in _program_cache:
        _program_cache[key] = build_program(**kw)
    return _program_cache[key]


def make_in_maps(x, ag_w, ag_b, lm_w, cfg=None):
    x = np.ascontiguousarray(np.asarray(x, dtype=np.float32))
    ag_w = np.asarray(ag_w, dtype=np.float32)
    ag_b = np.asarray(ag_b, dtype=np.float32)
    lm_w = np.asarray(lm_w, dtype=np.float32)

    agt = np.ascontiguousarray(
        ag_w.T.reshape(KT, 128, E).transpose(1, 0, 2).astype(np.float16)
    )
    negb = np.ascontiguousarray(
        np.broadcast_to(-ag_b[None, :], (128, E)).astype(np.float32)
    )
    # wfin[dp, gh, kt, gl, c, el] = lm_w[((gh*8+gl)*EG+el)*C+c, kt*128+dp]
    wfin = np.ascontiguousarray(
        lm_w.T.reshape(KT, 128, 2, NG // 2, EG, C)
        .transpose(1, 2, 0, 3, 5, 4)
        .astype(np.float16)
    )
    ep = np.arange(128) % EG
    mask = np.ascontiguousarray(
        (ep[:, None, None] == np.arange(EG)[None, None, :])
        * np.ones((128, C, EG), dtype=np.float16)
    )
    bidx = np.arange(128) // EG
    sel3 = (bidx[:, None] == np.arange(BL)[None, :]).astype(np.float16)

    common = {"agt": agt, "negb": negb, "wfin": wfin, "mask": mask,
              "sel3": sel3}
    in_maps = []
    for i in range(NCORES):
        xs = x[i * BL:(i + 1) * BL].reshape(T, D)
        m = dict(common)
        m["xn"] = np.ascontiguousarray(xs.astype(np.float16))
        # xt[dp, b, kt, h] = xs[b*HW+h, kt*128+dp]
        m["xt"] = np.ascontiguousarray(
            xs.T.reshape(KT, 128, BL, HW).transpose(1, 2, 0, 3)
            .astype(np.float16)
        )
        in_maps.append(m)
    return in_maps


def kernel(x, ag_w, ag_b, lm_w):
    in_maps = make_in_maps(x, ag_w, ag_b, lm_w)
    nc = _get_program()
    res = run_bass_kernel_spmd(nc, in_maps, core_ids=list(range(NCORES)))
    preds = np.concatenate(
        [res.results[i]["preds_o"] for i in range(NCORES)], axis=0
    )
    return np.ascontiguousarray(preds.astype(np.float32))


# revision 11
# speedup vs baseline: 1.8234x; 1.1005x over previous
"""Trainium2 Bass kernel for nn_ConvexMLPBlock.

Reference computation (B=64, HW=196, D=768, E=256, C=10):
    S[b,h,e]  = (x[b,h,:] @ ag_w[e,:] + ag_b[e]) > 0          (sign patterns)
    z[b,h,p]  = x[b,h,:] @ lm_w[p,:]        (p = e*C + c)
    preds[b,c] = sum_{h,e} S[b,h,e] * z[b,h,e,c] / (HW*E)

Restructured to avoid materializing z (49 GFLOP -> ~10 GFLOP):
    G_b[e,d]   = sum_h S[b,h,e] * x[b,h,d]                    (per-batch masked moment)
    preds[b,c] = (1/(HW*E)) * sum_{e,d} G_b[e,d] * W[e,c,d]   (W = lm_w.reshape(E,C,D))

Sharding: data-parallel over B across the 8 NeuronCores (8 batches/core);
host concatenates the per-core (8,10) outputs.

Per-core pipeline (v3):
    mm1: S[t,e] directly (stationary = x^T d-chunks, moving = ag^T [d,256]),
         ONE fp16 pass (rel err ~1.0e-2 < 2e-2 gate; fp16 products are exact
         in the PE, error comes only from operand rounding). No transposes.
    threshold: DVE tensor_tensor is_gt vs a broadcast (-ag_b) tile.
    mm2: G^T_b[d,e] contraction over h (stationary = x natural d-slices,
         moving = S), 2 h-tiles per batch, fp16.
    final: the e-diagonal selection mask is group-independent, so ALL 96
           cross-product matmuls (per d-tile and e-group: stationary
           G^T[d,(b,e)], moving W[d,(c,e)]) accumulate into a single
           [128,(c,e)] PSUM tile; then one mask-mult (DVE), one sel3
           partition-sum matmul, one e-reduce + scale.
    All DMAs ride the two HWDGE rings (SP + ACT) with per-partition
    contiguous layouts; SWDGE (gpsimd) is avoided entirely.
"""

import numpy as np

import concourse.bass as bass
import concourse.mybir as mybir
import concourse.tile as tile
from concourse.tile import add_dep_helper
from concourse.bass_utils import run_bass_kernel_spmd

# Problem constants (hardcoded per contract).
B = 64
HW = 196
D = 768
E = 256
C = 10
NCORES = 8
BL = B // NCORES          # local batches per core = 8
T = BL * HW               # local tokens = 1568
KT = D // 128             # 6 d-tiles
EG = 16                   # e's per final-stage group
NG = E // EG              # 16 groups

FP32 = mybir.dt.float32
BF16 = mybir.dt.bfloat16
FP16 = mybir.dt.float16


def _patched_drain_and_barrier(self, tick_clock, wait_clock):
    """This toolchain's walrus rejects >1 sync-wait on CTRL-class (Drain)
    instructions. Split the tail drain's global-clock waits across multiple
    single-wait drains. Semantics preserved: SP observes every DMA-queue
    semaphore before the all-engine barrier."""
    drain_inst = self.nc.sync.drain()
    wait_clock.add_sem_waits(
        drain_inst.ins, tile.ScopedClock({None: tick_clock.global_clock})
    )
    si = drain_inst.ins.sync_info
    if si is not None and si.on_wait is not None and len(si.on_wait) > 1:
        waits = list(si.on_wait)
        drain_inst.ins.sync_info = mybir.SyncInfo(
            on_wait=[waits[0]], on_update=list(si.on_update or [])
        )
        for w in waits[1:]:
            extra = self.nc.sync.drain()
            extra.ins.sync_info = mybir.SyncInfo(on_wait=[w], on_update=[])

    self.nc.all_engine_barrier()
    assert self.sems is not None
    popped = self.nc._tile_sem_poison_stack.pop()
    assert popped is self._sem_poison
    self.nc.clear_and_free_semaphores(list(self.sems.allocated().values()))
    self.nc.all_engine_barrier()


tile.TileContext._drain_and_barrier = _patched_drain_and_barrier


def _split_multiwait_json(bj: bytes) -> bytes:
    """Walrus in this toolchain accepts at most one sync-wait per instruction.
    For any instruction with N>1 waits, hoist N-1 waits onto same-engine NoOps
    inserted immediately before it. Engines execute program-order, so for
    compute instructions this is semantically identical; for DMAs it
    conservatively blocks the issuing engine instead of the queue."""
    import json

    m = json.loads(bj)
    changed = False
    for fn in m["functions"]:
        for bb in fn["blocks"]:
            new_insts = []
            for inst in bb["instructions"]:
                si = inst.get("sync_info")
                ow = (si or {}).get("on_wait") or []
                if len(ow) > 1:
                    for j, w in enumerate(ow[:-1]):
                        new_insts.append(
                            {
                                "name": f"{inst['name']}__w{j}",
                                "opcode": "NoOp",
                                "engine": inst["engine"],
                                "ins": [],
                                "outs": [],
                                "sync_info": {"on_update": [], "on_wait": [w]},
                            }
                        )
                    si["on_wait"] = [ow[-1]]
                    changed = True
                new_insts.append(inst)
            bb["instructions"] = new_insts
    if not changed:
        return bj
    return json.dumps(m).encode()


_orig_to_json_bytes = bass.Bass.to_json_bytes


def _patched_to_json_bytes(self, *a, **k):
    return _split_multiwait_json(_orig_to_json_bytes(self, *a, **k))


bass.Bass.to_json_bytes = _patched_to_json_bytes


# (batch, half) chunks: per batch a 128-row and a 68-row h-chunk.
CHUNKS = []
for _b in range(BL):
    CHUNKS.append((_b, 0, 0, 128))
    CHUNKS.append((_b, 1, 128, HW - 128))


def build_program(phases=("mm1", "mm2", "fin")):
    nc = bass.Bass()

    # xt[dp, b, kt, h] = x_core[b*HW+h, kt*128+dp]   (fp16, mm1 stationary)
    xt_d = nc.dram_tensor("xt", (128, BL, KT, HW), FP16,
                          kind="ExternalInput").ap()
    # agt[dp, kt, e] = ag_w[e, kt*128+dp]            (fp16, mm1 moving)
    agt_d = nc.dram_tensor("agt", (128, KT, E), FP16, kind="ExternalInput").ap()
    # negb[p, e] = -ag_b[e]                          (fp32, threshold)
    negb_d = nc.dram_tensor("negb", (128, E), FP32, kind="ExternalInput").ap()
    # xn_pk[p, ch, d] = x_core[chunk ch row p, d] (chunk-packed, tail-padded)
    xn_d = nc.dram_tensor("xn", (128, 2 * BL, D), FP16,
                          kind="ExternalInput").ap()
    # wfin[dp, gh, kt, gl, c, el] = lm_w[((gh*8+gl)*EG+el)*C+c, kt*128+dp]
    wfin_d = nc.dram_tensor("wfin", (128, 2, KT, NG // 2, C, EG), FP16,
                            kind="ExternalInput").ap()
    # mask[b*EG+ep, c, el] = (ep == el)
    mask_d = nc.dram_tensor("mask", (128, C, EG), FP16,
                            kind="ExternalInput").ap()
    # sel3[b*EG+ep, bp] = (b == bp)
    sel3_d = nc.dram_tensor("sel3", (128, BL), FP16, kind="ExternalInput").ap()
    preds_o = nc.dram_tensor("preds_o", (BL, C), FP32, kind="ExternalOutput").ap()

    from contextlib import ExitStack
    with tile.TileContext(nc) as tc, ExitStack() as _es:
        xt_p = _es.enter_context(tc.tile_pool(name="xt_p", bufs=1))
        agt_p = _es.enter_context(tc.tile_pool(name="agt_p", bufs=1))
        small_p = _es.enter_context(tc.tile_pool(name="small_p", bufs=1))
        sn_p = _es.enter_context(tc.tile_pool(name="sn_p", bufs=1))
        xn_p = _es.enter_context(tc.tile_pool(name="xn_p", bufs=1))
        gt_p = _es.enter_context(tc.tile_pool(name="gt_p", bufs=1))
        wfin_p = _es.enter_context(tc.tile_pool(name="wfin_p", bufs=1))
        out_p = _es.enter_context(tc.tile_pool(name="out_p", bufs=1))
        ps1 = _es.enter_context(tc.tile_pool(name="ps1", bufs=2, space="PSUM"))
        ps2 = _es.enter_context(tc.tile_pool(name="ps2", bufs=3, space="PSUM"))
        psM = _es.enter_context(tc.tile_pool(name="psM", bufs=1, space="PSUM"))
        psF = _es.enter_context(tc.tile_pool(name="psF", bufs=1, space="PSUM"))

        # ---- PE warm-up: HAM releases the PE clock gate (1.2 -> 2.4 GHz)
        # only after ~3.4us of sustained matmul activity; the first few us
        # are DMA-bound. Memsets ride DVE so the warm matmuls start at ~0.
        warm_src = small_p.tile([128, E], FP16, tag="warm_src",
                                name="warm_src")
        nc.vector.memset(warm_src[:], 0.0)
        warm_w = small_p.tile([128, 128], FP16, tag="warm_w", name="warm_w")
        nc.vector.memset(warm_w[:], 0.0)
        for wi in range(20):
            wps = ps1.tile([128, E], FP32, tag="ps1", name=f"warm_ps{wi}")
            nc.tensor.matmul(
                wps[:], warm_w[:], warm_src[:], start=True, stop=True
            )

        # ---- persistent loads. SP ring: mm1-critical stream + wfinA, in
        # consumption order, no deps (issue is ~0.65us each, serialized per
        # ring). ACT ring: small tensors only (it also runs gt copies).
        # SWDGE (gpsimd): the deferred streams (xn, wfinB) behind sem-wait
        # staggers -- blocking that queue is harmless.
        agt_sb = agt_p.tile([128, KT, E], FP16, tag="agt", name="agt_sb")
        nc.sync.dma_start(agt_sb[:], agt_d[:, :, :])
        xt_sb = xt_p.tile([128, BL, KT, HW], FP16, tag="xt", name="xt_sb")
        for bp in range(4):
            nc.sync.dma_start(xt_sb[:, 2 * bp:2 * bp + 2, :, :],
                              xt_d[:, 2 * bp:2 * bp + 2, :, :])

        mask_sb = small_p.tile([128, C, EG], FP16, tag="mask", name="mask_sb")
        nc.scalar.dma_start(mask_sb[:], mask_d[:, :, :])
        sel3_sb = small_p.tile([128, BL], FP16, tag="sel3", name="sel3_sb")
        nc.scalar.dma_start(sel3_sb[:], sel3_d[:, :])
        negb_sb = small_p.tile([128, E], FP32, tag="negb", name="negb_sb")
        nc.scalar.dma_start(negb_sb[:], negb_d[:, :])
        # Pre-load the ACT op table (~1.3us, one-time) during the DMA phase
        # so the first real nc.scalar.copy doesn't stall the gt pipeline.
        act_warm = small_p.tile([128, 8], FP16, tag="act_warm",
                                name="act_warm")
        nc.scalar.copy(act_warm[:], warm_w[:, 0:8])

        # ---- mm1: S[t,e] = (x @ ag_w^T > -b), single fp16 pass ----
        sn_sb = [
            sn_p.tile([128, E], FP16, tag=f"sn{ch}", name=f"sn_sb{ch}")
            for ch in range(len(CHUNKS))
        ]
        th_insts = {}
        for ch, (b, ht, h0, w) in enumerate(CHUNKS if "mm1" in phases else []):
            ps = ps1.tile([128, E], FP32, tag="ps1", name=f"ps1_{ch}")
            for kt in range(KT):
                nc.tensor.matmul(
                    ps[0:w, :],
                    xt_sb[:, b, kt, h0:h0 + w],
                    agt_sb[:, kt, :],
                    start=(kt == 0),
                    stop=(kt == KT - 1),
                )
            th_insts[ch] = nc.vector.tensor_tensor(
                sn_sb[ch][0:w, :], ps[0:w, :], negb_sb[0:w, :],
                mybir.AluOpType.is_gt,
            )

        # wfinA rides the SP ring after xt (no dep -- it transfers while mm1
        # computes); xn halves + wfinB ride SWDGE behind mm1-progress deps.
        wfin_sb = wfin_p.tile([128, 2, KT, NG // 2, C, EG], FP16, tag="wfin",
                              name="wfin_sb")
        nc.sync.dma_start(wfin_sb[:, 0, :, :, :, :], wfin_d[:, 0, :, :, :, :])

        xn_sb = xn_p.tile([128, 2 * BL, D], FP16, tag="xn", name="xn_sb")
        for xh, chd in ((0, 0), (1, 6)):
            dma = nc.gpsimd.dma_start(
                xn_sb[:, xh * BL:(xh + 1) * BL, :],
                xn_d[:, xh * BL:(xh + 1) * BL, :],
            )
            th = th_insts.get(chd)
            if th is not None:
                add_dep_helper(dma.ins, th.ins,
                               reason="stagger xn behind mm1")
        dma = nc.gpsimd.dma_start(wfin_sb[:, 1, :, :, :, :],
                                  wfin_d[:, 1, :, :, :, :])
        th = th_insts.get(10)
        if th is not None:
            add_dep_helper(dma.ins, th.ins,
                           reason="wfinB load after mm1 underway")

        # ---- mm2: G^T_b[d, e] = sum_h x[h,d] S[h,e] ----
        # grouped layout: gt[dt][dp, g, b, el] = G^T_b[dt*128+dp, g*EG+el]
        gt_sb = [
            gt_p.tile([128, NG, BL, EG], FP16, tag=f"gt{dt}",
                      name=f"gt_sb{dt}")
            for dt in range(KT)
        ]
        for b in range(BL if "mm2" in phases else 0):
            for dt in range(KT):
                pg = ps2.tile([128, E], FP32, tag="ps2", name=f"ps2_{b}_{dt}")
                for ht in range(2):
                    ch = 2 * b + ht
                    w = CHUNKS[ch][3]
                    nc.tensor.matmul(
                        pg[:],
                        xn_sb[0:w, ch, dt * 128:(dt + 1) * 128],
                        sn_sb[ch][0:w, :],
                        start=(ht == 0),
                        stop=(ht == 1),
                    )
                if (b + dt) % 2 == 0:
                    nc.vector.tensor_copy(gt_sb[dt][:, :, b, :], pg[:])
                else:
                    nc.scalar.copy(gt_sb[dt][:, :, b, :], pg[:])

        # ---- final ----
        # mask (e-diagonal selection) is identical for every group, so it
        # commutes with the group sum: ALL cross-product matmuls accumulate
        # into one PSUM tile, masked once at the end.
        do_fin = "fin" in phases
        pm = psM.tile([128, C, EG], FP32, tag="psM", name="psM_t")
        if not do_fin:
            nc.vector.memset(pm[:], 0.0)
        nmm = KT * NG
        im = 0
        for g in range(NG if do_fin else 0):
            gh, gl = g // (NG // 2), g % (NG // 2)
            for dt in range(KT):
                nc.tensor.matmul(
                    pm[:],
                    gt_sb[dt][:, g, :, :],
                    wfin_sb[:, gh, dt, gl, :, :],
                    start=(im == 0),
                    stop=(im == nmm - 1),
                )
                im += 1
        msb = out_p.tile([128, C, EG], FP16, tag="msb", name="msb_t")
        nc.vector.tensor_tensor(
            msb[:], pm[:], mask_sb[:], mybir.AluOpType.mult
        )
        pf = psF.tile([BL, C, EG], FP32, tag="psf", name="psf_t")
        nc.tensor.matmul(pf[:], sel3_sb[:], msb[:], start=True, stop=True)

        # final tiny reduction over e + scale
        red_sb = out_p.tile([BL, C], FP32, tag="red", name="red_sb")
        nc.vector.tensor_reduce(
            red_sb[:], pf[:], mybir.AxisListType.X, mybir.AluOpType.add
        )
        out_sb = out_p.tile([BL, C], FP32, tag="out", name="out_sb")
        nc.vector.tensor_scalar_mul(out_sb[:], red_sb[:], 1.0 / (HW * E))
        nc.sync.dma_start(preds_o[:, :], out_sb[:])

    return nc


_program_cache = {}

CONFIG = {}


def _get_program(**kw):
    key = tuple(sorted(kw.items()))
    if key not in _program_cache:
        _program_cache[key] = build_program(**kw)
    return _program_cache[key]


def make_in_maps(x, ag_w, ag_b, lm_w, cfg=None):
    x = np.ascontiguousarray(np.asarray(x, dtype=np.float32))
    ag_w = np.asarray(ag_w, dtype=np.float32)
    ag_b = np.asarray(ag_b, dtype=np.float32)
    lm_w = np.asarray(lm_w, dtype=np.float32)

    agt = np.ascontiguousarray(
        ag_w.T.reshape(KT, 128, E).transpose(1, 0, 2).astype(np.float16)
    )
    negb = np.ascontiguousarray(
        np.broadcast_to(-ag_b[None, :], (128, E)).astype(np.float32)
    )
    # wfin[dp, gh, kt, gl, c, el] = lm_w[((gh*8+gl)*EG+el)*C+c, kt*128+dp]
    wfin = np.ascontiguousarray(
        lm_w.T.reshape(KT, 128, 2, NG // 2, EG, C)
        .transpose(1, 2, 0, 3, 5, 4)
        .astype(np.float16)
    )
    ep = np.arange(128) % EG
    mask = np.ascontiguousarray(
        (ep[:, None, None] == np.arange(EG)[None, None, :])
        * np.ones((128, C, EG), dtype=np.float16)
    )
    bidx = np.arange(128) // EG
    sel3 = (bidx[:, None] == np.arange(BL)[None, :]).astype(np.float16)

    common = {"agt": agt, "negb": negb, "wfin": wfin, "mask": mask,
              "sel3": sel3}
    in_maps = []
    for i in range(NCORES):
        xs = x[i * BL:(i + 1) * BL].reshape(T, D)
        m = dict(common)
        # xn_pk[p, (b,ht), d]: 128-row chunk + zero-padded 68-row tail chunk
        xn_pk = np.zeros((128, 2 * BL, D), dtype=np.float16)
        xsb = xs.reshape(BL, HW, D).astype(np.float16)
        for b in range(BL):
            xn_pk[:, 2 * b, :] = xsb[b, 0:128, :]
            xn_pk[0:HW - 128, 2 * b + 1, :] = xsb[b, 128:HW, :]
        m["xn"] = np.ascontiguousarray(xn_pk)
        # xt[dp, b, kt, h] = xs[b*HW+h, kt*128+dp]
        m["xt"] = np.ascontiguousarray(
            xs.T.reshape(KT, 128, BL, HW).transpose(1, 2, 0, 3)
            .astype(np.float16)
        )
        in_maps.append(m)
    return in_maps


def kernel(x, ag_w, ag_b, lm_w):
    in_maps = make_in_maps(x, ag_w, ag_b, lm_w)
    nc = _get_program()
    res = run_bass_kernel_spmd(nc, in_maps, core_ids=list(range(NCORES)))
    preds = np.concatenate(
        [res.results[i]["preds_o"] for i in range(NCORES)], axis=0
    )
    return np.ascontiguousarray(preds.astype(np.float32))


# revision 13
# speedup vs baseline: 1.8864x; 1.0346x over previous
"""Trainium2 Bass kernel for nn_ConvexMLPBlock.

Reference computation (B=64, HW=196, D=768, E=256, C=10):
    S[b,h,e]  = (x[b,h,:] @ ag_w[e,:] + ag_b[e]) > 0          (sign patterns)
    z[b,h,p]  = x[b,h,:] @ lm_w[p,:]        (p = e*C + c)
    preds[b,c] = sum_{h,e} S[b,h,e] * z[b,h,e,c] / (HW*E)

Restructured to avoid materializing z (49 GFLOP -> ~10 GFLOP):
    G_b[e,d]   = sum_h S[b,h,e] * x[b,h,d]                    (per-batch masked moment)
    preds[b,c] = (1/(HW*E)) * sum_{e,d} G_b[e,d] * W[e,c,d]   (W = lm_w.reshape(E,C,D))

Sharding: data-parallel over B across the 8 NeuronCores (8 batches/core);
host concatenates the per-core (8,10) outputs.

Per-core pipeline (v3):
    mm1: S[t,e] directly (stationary = x^T d-chunks, moving = ag^T [d,256]),
         ONE fp16 pass (rel err ~1.0e-2 < 2e-2 gate; fp16 products are exact
         in the PE, error comes only from operand rounding). No transposes.
    threshold: DVE tensor_tensor is_gt vs a broadcast (-ag_b) tile.
    mm2: G^T_b[d,e] contraction over h (stationary = x natural d-slices,
         moving = S), 2 h-tiles per batch, fp16.
    final: the e-diagonal selection mask is group-independent, so ALL 96
           cross-product matmuls (per d-tile and e-group: stationary
           G^T[d,(b,e)], moving W[d,(c,e)]) accumulate into a single
           [128,(c,e)] PSUM tile; then one mask-mult (DVE), one sel3
           partition-sum matmul, one e-reduce + scale.
    All DMAs ride the two HWDGE rings (SP + ACT) with per-partition
    contiguous layouts; SWDGE (gpsimd) is avoided entirely.
"""

import numpy as np

import concourse.bass as bass
import concourse.mybir as mybir
import concourse.tile as tile
from concourse.tile import add_dep_helper
from concourse.bass_utils import run_bass_kernel_spmd

# Problem constants (hardcoded per contract).
B = 64
HW = 196
D = 768
E = 256
C = 10
NCORES = 8
BL = B // NCORES          # local batches per core = 8
T = BL * HW               # local tokens = 1568
KT = D // 128             # 6 d-tiles
EG = 16                   # e's per final-stage group
NG = E // EG              # 16 groups

FP32 = mybir.dt.float32
BF16 = mybir.dt.bfloat16
FP16 = mybir.dt.float16


def _patched_drain_and_barrier(self, tick_clock, wait_clock):
    """This toolchain's walrus rejects >1 sync-wait on CTRL-class (Drain)
    instructions. Split the tail drain's global-clock waits across multiple
    single-wait drains. Semantics preserved: SP observes every DMA-queue
    semaphore before the all-engine barrier."""
    drain_inst = self.nc.sync.drain()
    wait_clock.add_sem_waits(
        drain_inst.ins, tile.ScopedClock({None: tick_clock.global_clock})
    )
    si = drain_inst.ins.sync_info
    if si is not None and si.on_wait is not None and len(si.on_wait) > 1:
        waits = list(si.on_wait)
        drain_inst.ins.sync_info = mybir.SyncInfo(
            on_wait=[waits[0]], on_update=list(si.on_update or [])
        )
        for w in waits[1:]:
            extra = self.nc.sync.drain()
            extra.ins.sync_info = mybir.SyncInfo(on_wait=[w], on_update=[])

    self.nc.all_engine_barrier()
    assert self.sems is not None
    popped = self.nc._tile_sem_poison_stack.pop()
    assert popped is self._sem_poison
    self.nc.clear_and_free_semaphores(list(self.sems.allocated().values()))
    self.nc.all_engine_barrier()


tile.TileContext._drain_and_barrier = _patched_drain_and_barrier


def _split_multiwait_json(bj: bytes) -> bytes:
    """Walrus in this toolchain accepts at most one sync-wait per instruction.
    For any instruction with N>1 waits, hoist N-1 waits onto same-engine NoOps
    inserted immediately before it. Engines execute program-order, so for
    compute instructions this is semantically identical; for DMAs it
    conservatively blocks the issuing engine instead of the queue."""
    import json

    m = json.loads(bj)
    changed = False
    for fn in m["functions"]:
        for bb in fn["blocks"]:
            new_insts = []
            for inst in bb["instructions"]:
                si = inst.get("sync_info")
                ow = (si or {}).get("on_wait") or []
                if len(ow) > 1:
                    for j, w in enumerate(ow[:-1]):
                        new_insts.append(
                            {
                                "name": f"{inst['name']}__w{j}",
                                "opcode": "NoOp",
                                "engine": inst["engine"],
                                "ins": [],
                                "outs": [],
                                "sync_info": {"on_update": [], "on_wait": [w]},
                            }
                        )
                    si["on_wait"] = [ow[-1]]
                    changed = True
                new_insts.append(inst)
            bb["instructions"] = new_insts
    if not changed:
        return bj
    return json.dumps(m).encode()


_orig_to_json_bytes = bass.Bass.to_json_bytes


def _patched_to_json_bytes(self, *a, **k):
    return _split_multiwait_json(_orig_to_json_bytes(self, *a, **k))


bass.Bass.to_json_bytes = _patched_to_json_bytes


# (batch, half) chunks: per batch a 128-row and a 68-row h-chunk.
CHUNKS = []
for _b in range(BL):
    CHUNKS.append((_b, 0, 0, 128))
    CHUNKS.append((_b, 1, 128, HW - 128))


def build_program(phases=("mm1", "mm2", "fin")):
    nc = bass.Bass()

    # xt[dp, b, kt, h] = x_core[b*HW+h, kt*128+dp]   (fp16, mm1 stationary)
    xt_d = nc.dram_tensor("xt", (128, BL, KT, HW), FP16,
                          kind="ExternalInput").ap()
    # agt[dp, kt, e] = ag_w[e, kt*128+dp]            (fp16, mm1 moving)
    agt_d = nc.dram_tensor("agt", (128, KT, E), FP16, kind="ExternalInput").ap()
    # negb[p, e] = -ag_b[e]                          (fp32, threshold)
    negb_d = nc.dram_tensor("negb", (128, E), FP32, kind="ExternalInput").ap()
    # xn_pk[p, ch, d] = x_core[chunk ch row p, d] (chunk-packed, tail-padded)
    xn_d = nc.dram_tensor("xn", (128, 2 * BL, D), FP16,
                          kind="ExternalInput").ap()
    # wfin[dp, gh, kt, gl, c, el] = lm_w[((gh*8+gl)*EG+el)*C+c, kt*128+dp]
    wfin_d = nc.dram_tensor("wfin", (128, 2, KT, NG // 2, C, EG), FP16,
                            kind="ExternalInput").ap()
    # mask[b*EG+ep, c, el] = (ep == el)
    mask_d = nc.dram_tensor("mask", (128, C, EG), FP16,
                            kind="ExternalInput").ap()
    # sel3[b*EG+ep, bp] = (b == bp)
    sel3_d = nc.dram_tensor("sel3", (128, BL), FP16, kind="ExternalInput").ap()
    preds_o = nc.dram_tensor("preds_o", (BL, C), FP32, kind="ExternalOutput").ap()

    from contextlib import ExitStack
    with tile.TileContext(nc) as tc, ExitStack() as _es:
        xt_p = _es.enter_context(tc.tile_pool(name="xt_p", bufs=1))
        agt_p = _es.enter_context(tc.tile_pool(name="agt_p", bufs=1))
        small_p = _es.enter_context(tc.tile_pool(name="small_p", bufs=1))
        sn_p = _es.enter_context(tc.tile_pool(name="sn_p", bufs=1))
        xn_p = _es.enter_context(tc.tile_pool(name="xn_p", bufs=1))
        gt_p = _es.enter_context(tc.tile_pool(name="gt_p", bufs=1))
        wfin_p = _es.enter_context(tc.tile_pool(name="wfin_p", bufs=1))
        out_p = _es.enter_context(tc.tile_pool(name="out_p", bufs=1))
        ps1 = _es.enter_context(tc.tile_pool(name="ps1", bufs=2, space="PSUM"))
        ps2 = _es.enter_context(tc.tile_pool(name="ps2", bufs=3, space="PSUM"))
        psM = _es.enter_context(tc.tile_pool(name="psM", bufs=1, space="PSUM"))
        psF = _es.enter_context(tc.tile_pool(name="psF", bufs=1, space="PSUM"))

        # ---- PE warm-up: HAM releases the PE clock gate (1.2 -> 2.4 GHz)
        # only after ~3.4us of sustained matmul activity; the first few us
        # are DMA-bound. Memsets ride DVE so the warm matmuls start at ~0.
        warm_src = small_p.tile([128, E], FP16, tag="warm_src",
                                name="warm_src")
        nc.vector.memset(warm_src[:], 0.0)
        warm_w = small_p.tile([128, 128], FP16, tag="warm_w", name="warm_w")
        nc.vector.memset(warm_w[:], 0.0)
        for wi in range(14):
            wps = ps1.tile([128, E], FP32, tag="ps1", name=f"warm_ps{wi}")
            nc.tensor.matmul(
                wps[:], warm_w[:], warm_src[:], start=True, stop=True
            )

        # ---- persistent loads. SP ring carries the compute-critical stream
        # in consumption order (issue is ~0.65us each, serialized per ring):
        # agt, then alternating xt/xn batch-pairs, then wfinA. ACT ring:
        # small tensors only (it also runs gt copies). SWDGE (gpsimd):
        # wfinB behind an mm1-progress stagger (blocking it is harmless).
        agt_sb = agt_p.tile([128, KT, E], FP16, tag="agt", name="agt_sb")
        nc.sync.dma_start(agt_sb[:], agt_d[:, :, :])
        xt_sb = xt_p.tile([128, BL, KT, HW], FP16, tag="xt", name="xt_sb")
        xn_sb = xn_p.tile([128, 2 * BL, D], FP16, tag="xn", name="xn_sb")
        for bp in range(4):
            nc.sync.dma_start(xt_sb[:, 2 * bp:2 * bp + 2, :, :],
                              xt_d[:, 2 * bp:2 * bp + 2, :, :])
            nc.sync.dma_start(xn_sb[:, 4 * bp:4 * bp + 4, :],
                              xn_d[:, 4 * bp:4 * bp + 4, :])

        mask_sb = small_p.tile([128, C, EG], FP16, tag="mask", name="mask_sb")
        nc.scalar.dma_start(mask_sb[:], mask_d[:, :, :])
        sel3_sb = small_p.tile([128, BL], FP16, tag="sel3", name="sel3_sb")
        nc.scalar.dma_start(sel3_sb[:], sel3_d[:, :])
        negb_sb = small_p.tile([128, E], FP32, tag="negb", name="negb_sb")
        nc.scalar.dma_start(negb_sb[:], negb_d[:, :])
        # Pre-load the ACT op table (~1.3us, one-time) during the DMA phase
        # so the first real nc.scalar.copy doesn't stall the gt pipeline.
        act_warm = small_p.tile([128, 8], FP16, tag="act_warm",
                                name="act_warm")
        nc.scalar.copy(act_warm[:], warm_w[:, 0:8])

        # wfinA rides the SP ring right after the xt/xn stream (no dep --
        # it transfers while mm1/mm2 compute).
        wfin_sb = wfin_p.tile([128, 2, KT, NG // 2, C, EG], FP16, tag="wfin",
                              name="wfin_sb")
        nc.sync.dma_start(wfin_sb[:, 0, :, :, :, :], wfin_d[:, 0, :, :, :, :])

        # ---- mm1 + mm2, interleaved per batch pair so the PE stream is
        # ~2x denser than the xt/xn DMA stream (PE-bound, HAM stays warm).
        # mm1: S[t,e] = (x @ ag_w^T > -b), single fp16 pass.
        # mm2: G^T_b[d, e] = sum_h x[h,d] S[h,e];
        #      gt[dt][dp, g, b, el] = G^T_b[dt*128+dp, g*EG+el]
        sn_sb = [
            sn_p.tile([128, E], FP16, tag=f"sn{ch}", name=f"sn_sb{ch}")
            for ch in range(len(CHUNKS))
        ]
        gt_sb = [
            gt_p.tile([128, NG, BL, EG], FP16, tag=f"gt{dt}",
                      name=f"gt_sb{dt}")
            for dt in range(KT)
        ]
        th_insts = {}

        def emit_mm1(b):
            for ht in range(2):
                ch = 2 * b + ht
                _, _, h0, w = CHUNKS[ch]
                ps = ps1.tile([128, E], FP32, tag="ps1", name=f"ps1_{ch}")
                for kt in range(KT):
                    nc.tensor.matmul(
                        ps[0:w, :],
                        xt_sb[:, b, kt, h0:h0 + w],
                        agt_sb[:, kt, :],
                        start=(kt == 0),
                        stop=(kt == KT - 1),
                    )
                th_insts[ch] = nc.vector.tensor_tensor(
                    sn_sb[ch][0:w, :], ps[0:w, :], negb_sb[0:w, :],
                    mybir.AluOpType.is_gt,
                )

        def emit_mm2(b):
            for dt in range(KT):
                pg = ps2.tile([128, E], FP32, tag="ps2", name=f"ps2_{b}_{dt}")
                for ht in range(2):
                    ch = 2 * b + ht
                    w = CHUNKS[ch][3]
                    nc.tensor.matmul(
                        pg[:],
                        xn_sb[0:w, ch, dt * 128:(dt + 1) * 128],
                        sn_sb[ch][0:w, :],
                        start=(ht == 0),
                        stop=(ht == 1),
                    )
                if (b + dt) % 2 == 0:
                    nc.vector.tensor_copy(gt_sb[dt][:, :, b, :], pg[:])
                else:
                    nc.scalar.copy(gt_sb[dt][:, :, b, :], pg[:])

        for bp in range(4):
            emit_mm1(2 * bp)
            emit_mm1(2 * bp + 1)
            emit_mm2(2 * bp)
            emit_mm2(2 * bp + 1)
            if bp == 1:
                # wfinB on SWDGE once mm1 is underway; blocking the (other-
                # wise idle) gpsimd queue on this stagger is harmless.
                dma = nc.gpsimd.dma_start(wfin_sb[:, 1, :, :, :, :],
                                          wfin_d[:, 1, :, :, :, :])
                add_dep_helper(dma.ins, th_insts[2].ins,
                               reason="wfinB load after mm1 underway")

        # ---- final ----
        # mask (e-diagonal selection) is identical for every group, so it
        # commutes with the group sum: ALL cross-product matmuls accumulate
        # into one PSUM tile, masked once at the end.
        do_fin = "fin" in phases
        pm = psM.tile([128, C, EG], FP32, tag="psM", name="psM_t")
        if not do_fin:
            nc.vector.memset(pm[:], 0.0)
        nmm = KT * NG
        im = 0
        for g in range(NG if do_fin else 0):
            gh, gl = g // (NG // 2), g % (NG // 2)
            for dt in range(KT):
                nc.tensor.matmul(
                    pm[:],
                    gt_sb[dt][:, g, :, :],
                    wfin_sb[:, gh, dt, gl, :, :],
                    start=(im == 0),
                    stop=(im == nmm - 1),
                )
                im += 1
        msb = out_p.tile([128, C, EG], FP16, tag="msb", name="msb_t")
        nc.vector.tensor_tensor(
            msb[:], pm[:], mask_sb[:], mybir.AluOpType.mult
        )
        pf = psF.tile([BL, C, EG], FP32, tag="psf", name="psf_t")
        nc.tensor.matmul(pf[:], sel3_sb[:], msb[:], start=True, stop=True)

        # final tiny reduction over e + scale
        red_sb = out_p.tile([BL, C], FP32, tag="red", name="red_sb")
        nc.vector.tensor_reduce(
            red_sb[:], pf[:], mybir.AxisListType.X, mybir.AluOpType.add
        )
        out_sb = out_p.tile([BL, C], FP32, tag="out", name="out_sb")
        nc.vector.tensor_scalar_mul(out_sb[:], red_sb[:], 1.0 / (HW * E))
        nc.sync.dma_start(preds_o[:, :], out_sb[:])

    return nc


_program_cache = {}

CONFIG = {}


def _get_program(**kw):
    key = tuple(sorted(kw.items()))
    if key not in _program_cache:
        _program_cache[key] = build_program(**kw)
    return _program_cache[key]


def make_in_maps(x, ag_w, ag_b, lm_w, cfg=None):
    x = np.ascontiguousarray(np.asarray(x, dtype=np.float32))
    ag_w = np.asarray(ag_w, dtype=np.float32)
    ag_b = np.asarray(ag_b, dtype=np.float32)
    lm_w = np.asarray(lm_w, dtype=np.float32)

    agt = np.ascontiguousarray(
        ag_w.T.reshape(KT, 128, E).transpose(1, 0, 2).astype(np.float16)
    )
    negb = np.ascontiguousarray(
        np.broadcast_to(-ag_b[None, :], (128, E)).astype(np.float32)
    )
    # wfin[dp, gh, kt, gl, c, el] = lm_w[((gh*8+gl)*EG+el)*C+c, kt*128+dp]
    wfin = np.ascontiguousarray(
        lm_w.T.reshape(KT, 128, 2, NG // 2, EG, C)
        .transpose(1, 2, 0, 3, 5, 4)
        .astype(np.float16)
    )
    ep = np.arange(128) % EG
    mask = np.ascontiguousarray(
        (ep[:, None, None] == np.arange(EG)[None, None, :])
        * np.ones((128, C, EG), dtype=np.float16)
    )
    bidx = np.arange(128) // EG
    sel3 = (bidx[:, None] == np.arange(BL)[None, :]).astype(np.float16)

    common = {"agt": agt, "negb": negb, "wfin": wfin, "mask": mask,
              "sel3": sel3}
    in_maps = []
    for i in range(NCORES):
        xs = x[i * BL:(i + 1) * BL].reshape(T, D)
        m = dict(common)
        # xn_pk[p, (b,ht), d]: 128-row chunk + zero-padded 68-row tail chunk
        xn_pk = np.zeros((128, 2 * BL, D), dtype=np.float16)
        xsb = xs.reshape(BL, HW, D).astype(np.float16)
        for b in range(BL):
            xn_pk[:, 2 * b, :] = xsb[b, 0:128, :]
            xn_pk[0:HW - 128, 2 * b + 1, :] = xsb[b, 128:HW, :]
        m["xn"] = np.ascontiguousarray(xn_pk)
        # xt[dp, b, kt, h] = xs[b*HW+h, kt*128+dp]
        m["xt"] = np.ascontiguousarray(
            xs.T.reshape(KT, 128, BL, HW).transpose(1, 2, 0, 3)
            .astype(np.float16)
        )
        in_maps.append(m)
    return in_maps


def kernel(x, ag_w, ag_b, lm_w):
    in_maps = make_in_maps(x, ag_w, ag_b, lm_w)
    nc = _get_program()
    res = run_bass_kernel_spmd(nc, in_maps, core_ids=list(range(NCORES)))
    preds = np.concatenate(
        [res.results[i]["preds_o"] for i in range(NCORES)], axis=0
    )
    return np.ascontiguousarray(preds.astype(np.float32))


# revision 14
# speedup vs baseline: 1.8932x; 1.0036x over previous
"""Trainium2 Bass kernel for nn_ConvexMLPBlock.

Reference computation (B=64, HW=196, D=768, E=256, C=10):
    S[b,h,e]  = (x[b,h,:] @ ag_w[e,:] + ag_b[e]) > 0          (sign patterns)
    z[b,h,p]  = x[b,h,:] @ lm_w[p,:]        (p = e*C + c)
    preds[b,c] = sum_{h,e} S[b,h,e] * z[b,h,e,c] / (HW*E)

Restructured to avoid materializing z (49 GFLOP -> ~10 GFLOP):
    G_b[e,d]   = sum_h S[b,h,e] * x[b,h,d]                    (per-batch masked moment)
    preds[b,c] = (1/(HW*E)) * sum_{e,d} G_b[e,d] * W[e,c,d]   (W = lm_w.reshape(E,C,D))

Sharding: data-parallel over B across the 8 NeuronCores (8 batches/core);
host concatenates the per-core (8,10) outputs.

Per-core pipeline (v3):
    mm1: S[t,e] directly (stationary = x^T d-chunks, moving = ag^T [d,256]),
         ONE fp16 pass (rel err ~1.0e-2 < 2e-2 gate; fp16 products are exact
         in the PE, error comes only from operand rounding). No transposes.
    threshold: DVE tensor_tensor is_gt vs a broadcast (-ag_b) tile.
    mm2: G^T_b[d,e] contraction over h (stationary = x natural d-slices,
         moving = S), 2 h-tiles per batch, fp16.
    final: the e-diagonal selection mask is group-independent, so ALL 96
           cross-product matmuls (per d-tile and e-group: stationary
           G^T[d,(b,e)], moving W[d,(c,e)]) accumulate into a single
           [128,(c,e)] PSUM tile; then one mask-mult (DVE), one sel3
           partition-sum matmul, one e-reduce + scale.
    All DMAs ride the two HWDGE rings (SP + ACT) with per-partition
    contiguous layouts; SWDGE (gpsimd) is avoided entirely.
"""

import numpy as np

import concourse.bass as bass
import concourse.mybir as mybir
import concourse.tile as tile
from concourse.tile import add_dep_helper
from concourse.bass_utils import run_bass_kernel_spmd

# Problem constants (hardcoded per contract).
B = 64
HW = 196
D = 768
E = 256
C = 10
NCORES = 8
BL = B // NCORES          # local batches per core = 8
T = BL * HW               # local tokens = 1568
KT = D // 128             # 6 d-tiles
EG = 16                   # e's per final-stage group
NG = E // EG              # 16 groups

FP32 = mybir.dt.float32
BF16 = mybir.dt.bfloat16
FP16 = mybir.dt.float16


def _patched_drain_and_barrier(self, tick_clock, wait_clock):
    """This toolchain's walrus rejects >1 sync-wait on CTRL-class (Drain)
    instructions. Split the tail drain's global-clock waits across multiple
    single-wait drains. Semantics preserved: SP observes every DMA-queue
    semaphore before the all-engine barrier."""
    drain_inst = self.nc.sync.drain()
    wait_clock.add_sem_waits(
        drain_inst.ins, tile.ScopedClock({None: tick_clock.global_clock})
    )
    si = drain_inst.ins.sync_info
    if si is not None and si.on_wait is not None and len(si.on_wait) > 1:
        waits = list(si.on_wait)
        drain_inst.ins.sync_info = mybir.SyncInfo(
            on_wait=[waits[0]], on_update=list(si.on_update or [])
        )
        for w in waits[1:]:
            extra = self.nc.sync.drain()
            extra.ins.sync_info = mybir.SyncInfo(on_wait=[w], on_update=[])

    self.nc.all_engine_barrier()
    assert self.sems is not None
    popped = self.nc._tile_sem_poison_stack.pop()
    assert popped is self._sem_poison
    self.nc.clear_and_free_semaphores(list(self.sems.allocated().values()))
    self.nc.all_engine_barrier()


tile.TileContext._drain_and_barrier = _patched_drain_and_barrier


def _split_multiwait_json(bj: bytes) -> bytes:
    """Walrus in this toolchain accepts at most one sync-wait per instruction.
    For any instruction with N>1 waits, hoist N-1 waits onto same-engine NoOps
    inserted immediately before it. Engines execute program-order, so for
    compute instructions this is semantically identical; for DMAs it
    conservatively blocks the issuing engine instead of the queue."""
    import json

    m = json.loads(bj)
    changed = False
    for fn in m["functions"]:
        for bb in fn["blocks"]:
            new_insts = []
            for inst in bb["instructions"]:
                si = inst.get("sync_info")
                ow = (si or {}).get("on_wait") or []
                if len(ow) > 1:
                    for j, w in enumerate(ow[:-1]):
                        new_insts.append(
                            {
                                "name": f"{inst['name']}__w{j}",
                                "opcode": "NoOp",
                                "engine": inst["engine"],
                                "ins": [],
                                "outs": [],
                                "sync_info": {"on_update": [], "on_wait": [w]},
                            }
                        )
                    si["on_wait"] = [ow[-1]]
                    changed = True
                new_insts.append(inst)
            bb["instructions"] = new_insts
    if not changed:
        return bj
    return json.dumps(m).encode()


_orig_to_json_bytes = bass.Bass.to_json_bytes


def _patched_to_json_bytes(self, *a, **k):
    return _split_multiwait_json(_orig_to_json_bytes(self, *a, **k))


bass.Bass.to_json_bytes = _patched_to_json_bytes


# (batch, half) chunks: per batch a 128-row and a 68-row h-chunk.
CHUNKS = []
for _b in range(BL):
    CHUNKS.append((_b, 0, 0, 128))
    CHUNKS.append((_b, 1, 128, HW - 128))


def build_program(phases=("mm1", "mm2", "fin")):
    nc = bass.Bass()

    # xt[dp, b, kt, h] = x_core[b*HW+h, kt*128+dp]   (fp16, mm1 stationary)
    xt_d = nc.dram_tensor("xt", (128, BL, KT, HW), FP16,
                          kind="ExternalInput").ap()
    # agt[dp, kt, e] = ag_w[e, kt*128+dp]            (fp16, mm1 moving)
    agt_d = nc.dram_tensor("agt", (128, KT, E), FP16, kind="ExternalInput").ap()
    # negb[p, e] = -ag_b[e]                          (fp32, threshold)
    negb_d = nc.dram_tensor("negb", (128, E), FP32, kind="ExternalInput").ap()
    # xn_pk[p, ch, d] = x_core[chunk ch row p, d] (chunk-packed, tail-padded)
    xn_d = nc.dram_tensor("xn", (128, 2 * BL, D), FP16,
                          kind="ExternalInput").ap()
    # wfin[dp, gh, kt, gl, c, el] = lm_w[((gh*8+gl)*EG+el)*C+c, kt*128+dp]
    wfin_d = nc.dram_tensor("wfin", (128, 2, KT, NG // 2, C, EG), FP16,
                            kind="ExternalInput").ap()
    # mask[b*EG+ep, c, el] = (ep == el)
    mask_d = nc.dram_tensor("mask", (128, C, EG), FP16,
                            kind="ExternalInput").ap()
    # sel3[b*EG+ep, bp] = (b == bp)
    sel3_d = nc.dram_tensor("sel3", (128, BL), FP16, kind="ExternalInput").ap()
    preds_o = nc.dram_tensor("preds_o", (BL, C), FP32, kind="ExternalOutput").ap()

    from contextlib import ExitStack
    with tile.TileContext(nc) as tc, ExitStack() as _es:
        xt_p = _es.enter_context(tc.tile_pool(name="xt_p", bufs=1))
        agt_p = _es.enter_context(tc.tile_pool(name="agt_p", bufs=1))
        small_p = _es.enter_context(tc.tile_pool(name="small_p", bufs=1))
        sn_p = _es.enter_context(tc.tile_pool(name="sn_p", bufs=1))
        xn_p = _es.enter_context(tc.tile_pool(name="xn_p", bufs=1))
        gt_p = _es.enter_context(tc.tile_pool(name="gt_p", bufs=1))
        wfin_p = _es.enter_context(tc.tile_pool(name="wfin_p", bufs=1))
        out_p = _es.enter_context(tc.tile_pool(name="out_p", bufs=1))
        ps1 = _es.enter_context(tc.tile_pool(name="ps1", bufs=2, space="PSUM"))
        ps2 = _es.enter_context(tc.tile_pool(name="ps2", bufs=3, space="PSUM"))
        psM = _es.enter_context(tc.tile_pool(name="psM", bufs=1, space="PSUM"))
        psF = _es.enter_context(tc.tile_pool(name="psF", bufs=1, space="PSUM"))

        # ---- PE warm-up: HAM releases the PE clock gate (1.2 -> 2.4 GHz)
        # only after ~3.4us of sustained matmul activity; the first few us
        # are DMA-bound. Memsets ride DVE so the warm matmuls start at ~0.
        warm_src = small_p.tile([128, E], FP16, tag="warm_src",
                                name="warm_src")
        nc.vector.memset(warm_src[:], 0.0)
        warm_w = small_p.tile([128, 128], FP16, tag="warm_w", name="warm_w")
        nc.vector.memset(warm_w[:], 0.0)
        for wi in range(12):
            wps = ps1.tile([128, E], FP32, tag="ps1", name=f"warm_ps{wi}")
            nc.tensor.matmul(
                wps[:], warm_w[:], warm_src[:], start=True, stop=True
            )

        # ---- persistent loads, consumption order, issue ~0.65us each
        # serialized per ring. The first two tensors mm1 needs (agt, xt01)
        # ride DIFFERENT rings so they transfer concurrently; the rest of
        # the xt/xn stream alternates on SP. ACT ring: small tensors (it
        # also runs gt copies). SWDGE (gpsimd): wfinB behind a stagger.
        agt_sb = agt_p.tile([128, KT, E], FP16, tag="agt", name="agt_sb")
        nc.sync.dma_start(agt_sb[:], agt_d[:, :, :])
        xt_sb = xt_p.tile([128, BL, KT, HW], FP16, tag="xt", name="xt_sb")
        xn_sb = xn_p.tile([128, 2 * BL, D], FP16, tag="xn", name="xn_sb")
        nc.scalar.dma_start(xt_sb[:, 0:2, :, :], xt_d[:, 0:2, :, :])
        nc.sync.dma_start(xn_sb[:, 0:4, :], xn_d[:, 0:4, :])
        for bp in range(1, 4):
            nc.sync.dma_start(xt_sb[:, 2 * bp:2 * bp + 2, :, :],
                              xt_d[:, 2 * bp:2 * bp + 2, :, :])
            nc.sync.dma_start(xn_sb[:, 4 * bp:4 * bp + 4, :],
                              xn_d[:, 4 * bp:4 * bp + 4, :])

        mask_sb = small_p.tile([128, C, EG], FP16, tag="mask", name="mask_sb")
        nc.scalar.dma_start(mask_sb[:], mask_d[:, :, :])
        sel3_sb = small_p.tile([128, BL], FP16, tag="sel3", name="sel3_sb")
        nc.scalar.dma_start(sel3_sb[:], sel3_d[:, :])
        negb_sb = small_p.tile([128, E], FP32, tag="negb", name="negb_sb")
        nc.scalar.dma_start(negb_sb[:], negb_d[:, :])
        # Pre-load the ACT op table (~1.3us, one-time) during the DMA phase
        # so the first real nc.scalar.copy doesn't stall the gt pipeline.
        act_warm = small_p.tile([128, 8], FP16, tag="act_warm",
                                name="act_warm")
        nc.scalar.copy(act_warm[:], warm_w[:, 0:8])

        # wfinA rides the SP ring right after the xt/xn stream (no dep --
        # it transfers while mm1/mm2 compute).
        wfin_sb = wfin_p.tile([128, 2, KT, NG // 2, C, EG], FP16, tag="wfin",
                              name="wfin_sb")
        nc.sync.dma_start(wfin_sb[:, 0, :, :, :, :], wfin_d[:, 0, :, :, :, :])

        # ---- mm1 + mm2, interleaved per batch pair so the PE stream is
        # ~2x denser than the xt/xn DMA stream (PE-bound, HAM stays warm).
        # mm1: S[t,e] = (x @ ag_w^T > -b), single fp16 pass.
        # mm2: G^T_b[d, e] = sum_h x[h,d] S[h,e];
        #      gt[dt][dp, g, b, el] = G^T_b[dt*128+dp, g*EG+el]
        sn_sb = [
            sn_p.tile([128, E], FP16, tag=f"sn{ch}", name=f"sn_sb{ch}")
            for ch in range(len(CHUNKS))
        ]
        gt_sb = [
            gt_p.tile([128, NG, BL, EG], FP16, tag=f"gt{dt}",
                      name=f"gt_sb{dt}")
            for dt in range(KT)
        ]
        th_insts = {}

        def emit_mm1(b):
            for ht in range(2):
                ch = 2 * b + ht
                _, _, h0, w = CHUNKS[ch]
                ps = ps1.tile([128, E], FP32, tag="ps1", name=f"ps1_{ch}")
                for kt in range(KT):
                    nc.tensor.matmul(
                        ps[0:w, :],
                        xt_sb[:, b, kt, h0:h0 + w],
                        agt_sb[:, kt, :],
                        start=(kt == 0),
                        stop=(kt == KT - 1),
                    )
                th_insts[ch] = nc.vector.tensor_tensor(
                    sn_sb[ch][0:w, :], ps[0:w, :], negb_sb[0:w, :],
                    mybir.AluOpType.is_gt,
                )

        def emit_mm2(b):
            for dt in range(KT):
                pg = ps2.tile([128, E], FP32, tag="ps2", name=f"ps2_{b}_{dt}")
                for ht in range(2):
                    ch = 2 * b + ht
                    w = CHUNKS[ch][3]
                    nc.tensor.matmul(
                        pg[:],
                        xn_sb[0:w, ch, dt * 128:(dt + 1) * 128],
                        sn_sb[ch][0:w, :],
                        start=(ht == 0),
                        stop=(ht == 1),
                    )
                if (b + dt) % 2 == 0:
                    nc.vector.tensor_copy(gt_sb[dt][:, :, b, :], pg[:])
                else:
                    nc.scalar.copy(gt_sb[dt][:, :, b, :], pg[:])

        for bp in range(4):
            emit_mm1(2 * bp)
            emit_mm1(2 * bp + 1)
            emit_mm2(2 * bp)
            emit_mm2(2 * bp + 1)
            if bp == 1:
                # wfinB on SWDGE once mm1 is underway; blocking the (other-
                # wise idle) gpsimd queue on this stagger is harmless.
                dma = nc.gpsimd.dma_start(wfin_sb[:, 1, :, :, :, :],
                                          wfin_d[:, 1, :, :, :, :])
                add_dep_helper(dma.ins, th_insts[2].ins,
                               reason="wfinB load after mm1 underway")

        # ---- final ----
        # mask (e-diagonal selection) is identical for every group, so it
        # commutes with the group sum: ALL cross-product matmuls accumulate
        # into one PSUM tile, masked once at the end.
        do_fin = "fin" in phases
        pm = psM.tile([128, C, EG], FP32, tag="psM", name="psM_t")
        if not do_fin:
            nc.vector.memset(pm[:], 0.0)
        nmm = KT * NG
        im = 0
        for g in range(NG if do_fin else 0):
            gh, gl = g // (NG // 2), g % (NG // 2)
            for dt in range(KT):
                nc.tensor.matmul(
                    pm[:],
                    gt_sb[dt][:, g, :, :],
                    wfin_sb[:, gh, dt, gl, :, :],
                    start=(im == 0),
                    stop=(im == nmm - 1),
                )
                im += 1
        msb = out_p.tile([128, C, EG], FP16, tag="msb", name="msb_t")
        nc.vector.tensor_tensor(
            msb[:], pm[:], mask_sb[:], mybir.AluOpType.mult
        )
        pf = psF.tile([BL, C, EG], FP32, tag="psf", name="psf_t")
        nc.tensor.matmul(pf[:], sel3_sb[:], msb[:], start=True, stop=True)

        # final tiny reduction over e + scale
        red_sb = out_p.tile([BL, C], FP32, tag="red", name="red_sb")
        nc.vector.tensor_reduce(
            red_sb[:], pf[:], mybir.AxisListType.X, mybir.AluOpType.add
        )
        out_sb = out_p.tile([BL, C], FP32, tag="out", name="out_sb")
        nc.vector.tensor_scalar_mul(out_sb[:], red_sb[:], 1.0 / (HW * E))
        nc.sync.dma_start(preds_o[:, :], out_sb[:])

    return nc


_program_cache = {}

CONFIG = {}


def _get_program(**kw):
    key = tuple(sorted(kw.items()))
    if key not in _program_cache:
        _program_cache[key] = build_program(**kw)
    return _program_cache[key]


def make_in_maps(x, ag_w, ag_b, lm_w, cfg=None):
    x = np.ascontiguousarray(np.asarray(x, dtype=np.float32))
    ag_w = np.asarray(ag_w, dtype=np.float32)
    ag_b = np.asarray(ag_b, dtype=np.float32)
    lm_w = np.asarray(lm_w, dtype=np.float32)

    agt = np.ascontiguousarray(
        ag_w.T.reshape(KT, 128, E).transpose(1, 0, 2).astype(np.float16)
    )
    negb = np.ascontiguousarray(
        np.broadcast_to(-ag_b[None, :], (128, E)).astype(np.float32)
    )
    # wfin[dp, gh, kt, gl, c, el] = lm_w[((gh*8+gl)*EG+el)*C+c, kt*128+dp]
    wfin = np.ascontiguousarray(
        lm_w.T.reshape(KT, 128, 2, NG // 2, EG, C)
        .transpose(1, 2, 0, 3, 5, 4)
        .astype(np.float16)
    )
    ep = np.arange(128) % EG
    mask = np.ascontiguousarray(
        (ep[:, None, None] == np.arange(EG)[None, None, :])
        * np.ones((128, C, EG), dtype=np.float16)
    )
    bidx = np.arange(128) // EG
    sel3 = (bidx[:, None] == np.arange(BL)[None, :]).astype(np.float16)

    common = {"agt": agt, "negb": negb, "wfin": wfin, "mask": mask,
              "sel3": sel3}
    in_maps = []
    for i in range(NCORES):
        xs = x[i * BL:(i + 1) * BL].reshape(T, D)
        m = dict(common)
        # xn_pk[p, (b,ht), d]: 128-row chunk + zero-padded 68-row tail chunk
        xn_pk = np.zeros((128, 2 * BL, D), dtype=np.float16)
        xsb = xs.reshape(BL, HW, D).astype(np.float16)
        for b in range(BL):
            xn_pk[:, 2 * b, :] = xsb[b, 0:128, :]
            xn_pk[0:HW - 128, 2 * b + 1, :] = xsb[b, 128:HW, :]
        m["xn"] = np.ascontiguousarray(xn_pk)
        # xt[dp, b, kt, h] = xs[b*HW+h, kt*128+dp]
        m["xt"] = np.ascontiguousarray(
            xs.T.reshape(KT, 128, BL, HW).transpose(1, 2, 0, 3)
            .astype(np.float16)
        )
        in_maps.append(m)
    return in_maps


def kernel(x, ag_w, ag_b, lm_w):
    in_maps = make_in_maps(x, ag_w, ag_b, lm_w)
    nc = _get_program()
    res = run_bass_kernel_spmd(nc, in_maps, core_ids=list(range(NCORES)))
    preds = np.concatenate(
        [res.results[i]["preds_o"] for i in range(NCORES)], axis=0
    )
    return np.ascontiguousarray(preds.astype(np.float32))


# revision 16
# speedup vs baseline: 1.9512x; 1.0307x over previous
"""Trainium2 Bass kernel for nn_ConvexMLPBlock.

Reference computation (B=64, HW=196, D=768, E=256, C=10):
    S[b,h,e]  = (x[b,h,:] @ ag_w[e,:] + ag_b[e]) > 0          (sign patterns)
    z[b,h,p]  = x[b,h,:] @ lm_w[p,:]        (p = e*C + c)
    preds[b,c] = sum_{h,e} S[b,h,e] * z[b,h,e,c] / (HW*E)

Restructured to avoid materializing z (49 GFLOP -> ~10 GFLOP):
    G_b[e,d]   = sum_h S[b,h,e] * x[b,h,d]                    (per-batch masked moment)
    preds[b,c] = (1/(HW*E)) * sum_{e,d} G_b[e,d] * W[e,c,d]   (W = lm_w.reshape(E,C,D))

Sharding: data-parallel over B across the 8 NeuronCores (8 batches/core);
host concatenates the per-core (8,10) outputs.

Per-core pipeline (v3):
    mm1: S[t,e] directly (stationary = x^T d-chunks, moving = ag^T [d,256]),
         ONE fp16 pass (rel err ~1.0e-2 < 2e-2 gate; fp16 products are exact
         in the PE, error comes only from operand rounding). No transposes.
    threshold: DVE tensor_tensor is_gt vs a broadcast (-ag_b) tile.
    mm2: G^T_b[d,e] contraction over h (stationary = x natural d-slices,
         moving = S), 2 h-tiles per batch, fp16.
    final: the e-diagonal selection mask is group-independent, so ALL 96
           cross-product matmuls (per d-tile and e-group: stationary
           G^T[d,(b,e)], moving W[d,(c,e)]) accumulate into a single
           [128,(c,e)] PSUM tile; then one mask-mult (DVE), one sel3
           partition-sum matmul, one e-reduce + scale.
    All DMAs ride the two HWDGE rings (SP + ACT) with per-partition
    contiguous layouts; SWDGE (gpsimd) is avoided entirely.
"""

import numpy as np

import concourse.bass as bass
import concourse.mybir as mybir
import concourse.tile as tile
from concourse.tile import add_dep_helper
from concourse.bass_utils import run_bass_kernel_spmd

# Problem constants (hardcoded per contract).
B = 64
HW = 196
D = 768
E = 256
C = 10
NCORES = 8
BL = B // NCORES          # local batches per core = 8
T = BL * HW               # local tokens = 1568
KT = D // 128             # 6 d-tiles
EG = 16                   # e's per final-stage group
NG = E // EG              # 16 groups

FP32 = mybir.dt.float32
BF16 = mybir.dt.bfloat16
FP16 = mybir.dt.float16


def _patched_drain_and_barrier(self, tick_clock, wait_clock):
    """This toolchain's walrus rejects >1 sync-wait on CTRL-class (Drain)
    instructions. Split the tail drain's global-clock waits across multiple
    single-wait drains. Semantics preserved: SP observes every DMA-queue
    semaphore before the all-engine barrier."""
    drain_inst = self.nc.sync.drain()
    wait_clock.add_sem_waits(
        drain_inst.ins, tile.ScopedClock({None: tick_clock.global_clock})
    )
    si = drain_inst.ins.sync_info
    if si is not None and si.on_wait is not None and len(si.on_wait) > 1:
        waits = list(si.on_wait)
        drain_inst.ins.sync_info = mybir.SyncInfo(
            on_wait=[waits[0]], on_update=list(si.on_update or [])
        )
        for w in waits[1:]:
            extra = self.nc.sync.drain()
            extra.ins.sync_info = mybir.SyncInfo(on_wait=[w], on_update=[])

    self.nc.all_engine_barrier()
    assert self.sems is not None
    popped = self.nc._tile_sem_poison_stack.pop()
    assert popped is self._sem_poison
    self.nc.clear_and_free_semaphores(list(self.sems.allocated().values()))
    self.nc.all_engine_barrier()


tile.TileContext._drain_and_barrier = _patched_drain_and_barrier


def _split_multiwait_json(bj: bytes) -> bytes:
    """Walrus in this toolchain accepts at most one sync-wait per instruction.
    For any instruction with N>1 waits, hoist N-1 waits onto same-engine NoOps
    inserted immediately before it. Engines execute program-order, so for
    compute instructions this is semantically identical; for DMAs it
    conservatively blocks the issuing engine instead of the queue."""
    import json

    m = json.loads(bj)
    changed = False
    for fn in m["functions"]:
        for bb in fn["blocks"]:
            new_insts = []
            for inst in bb["instructions"]:
                si = inst.get("sync_info")
                ow = (si or {}).get("on_wait") or []
                if len(ow) > 1:
                    for j, w in enumerate(ow[:-1]):
                        new_insts.append(
                            {
                                "name": f"{inst['name']}__w{j}",
                                "opcode": "NoOp",
                                "engine": inst["engine"],
                                "ins": [],
                                "outs": [],
                                "sync_info": {"on_update": [], "on_wait": [w]},
                            }
                        )
                    si["on_wait"] = [ow[-1]]
                    changed = True
                new_insts.append(inst)
            bb["instructions"] = new_insts
    if not changed:
        return bj
    return json.dumps(m).encode()


_orig_to_json_bytes = bass.Bass.to_json_bytes


def _patched_to_json_bytes(self, *a, **k):
    return _split_multiwait_json(_orig_to_json_bytes(self, *a, **k))


bass.Bass.to_json_bytes = _patched_to_json_bytes


# (batch, half) chunks: per batch a 128-row and a 68-row h-chunk.
CHUNKS = []
for _b in range(BL):
    CHUNKS.append((_b, 0, 0, 128))
    CHUNKS.append((_b, 1, 128, HW - 128))


def build_program(phases=("mm1", "mm2", "fin")):
    nc = bass.Bass()

    # xt[dp, b, kt, h] = x_core[b*HW+h, kt*128+dp]   (fp16, mm1 stationary)
    xt_d = nc.dram_tensor("xt", (128, BL, KT, HW), FP16,
                          kind="ExternalInput").ap()
    # agt[dp, kt, e] = ag_w[e, kt*128+dp]            (fp16, mm1 moving)
    agt_d = nc.dram_tensor("agt", (128, KT, E), FP16, kind="ExternalInput").ap()
    # negb[p, e] = -ag_b[e]                          (fp32, threshold)
    negb_d = nc.dram_tensor("negb", (128, E), FP32, kind="ExternalInput").ap()
    # xn_pk[p, ch, d] = x_core[chunk ch row p, d] (chunk-packed, tail-padded)
    xn_d = nc.dram_tensor("xn", (128, 2 * BL, D), FP16,
                          kind="ExternalInput").ap()
    # wfin[dp, gh, kt, gl, c, el] = lm_w[((gh*8+gl)*EG+el)*C+c, kt*128+dp]
    wfin_d = nc.dram_tensor("wfin", (128, 2, KT, NG // 2, C, EG), FP16,
                            kind="ExternalInput").ap()
    # mask[b*EG+ep, c, el] = (ep == el)
    mask_d = nc.dram_tensor("mask", (128, C, EG), FP16,
                            kind="ExternalInput").ap()
    # sel3[b*EG+ep, bp] = (b == bp)
    sel3_d = nc.dram_tensor("sel3", (128, BL), FP16, kind="ExternalInput").ap()
    preds_o = nc.dram_tensor("preds_o", (BL, C), FP32, kind="ExternalOutput").ap()

    from contextlib import ExitStack
    with tile.TileContext(nc) as tc, ExitStack() as _es:
        xt_p = _es.enter_context(tc.tile_pool(name="xt_p", bufs=1))
        agt_p = _es.enter_context(tc.tile_pool(name="agt_p", bufs=1))
        small_p = _es.enter_context(tc.tile_pool(name="small_p", bufs=1))
        sn_p = _es.enter_context(tc.tile_pool(name="sn_p", bufs=1))
        xn_p = _es.enter_context(tc.tile_pool(name="xn_p", bufs=1))
        gt_p = _es.enter_context(tc.tile_pool(name="gt_p", bufs=1))
        wfin_p = _es.enter_context(tc.tile_pool(name="wfin_p", bufs=1))
        out_p = _es.enter_context(tc.tile_pool(name="out_p", bufs=1))
        ps1 = _es.enter_context(tc.tile_pool(name="ps1", bufs=3, space="PSUM"))
        ps2 = _es.enter_context(tc.tile_pool(name="ps2", bufs=3, space="PSUM"))
        psM = _es.enter_context(tc.tile_pool(name="psM", bufs=1, space="PSUM"))
        psF = _es.enter_context(tc.tile_pool(name="psF", bufs=1, space="PSUM"))

        # ---- PE warm-up: HAM releases the PE clock gate (1.2 -> 2.4 GHz)
        # only after ~3.4us of sustained matmul activity; the first few us
        # are DMA-bound. Memsets ride DVE so the warm matmuls start at ~0.
        warm_src = small_p.tile([128, E], FP16, tag="warm_src",
                                name="warm_src")
        nc.vector.memset(warm_src[:], 0.0)
        warm_w = small_p.tile([128, 128], FP16, tag="warm_w", name="warm_w")
        nc.vector.memset(warm_w[:], 0.0)
        for wi in range(12):
            wps = ps1.tile([128, E], FP32, tag="ps1", name=f"warm_ps{wi}")
            nc.tensor.matmul(
                wps[:], warm_w[:], warm_src[:], start=True, stop=True
            )

        # ---- persistent loads, consumption order, issue ~0.65us each
        # serialized per ring. The first two tensors mm1 needs (agt, xt01)
        # ride DIFFERENT rings so they transfer concurrently; the rest of
        # the xt/xn stream alternates on SP. ACT ring: small tensors (it
        # also runs gt copies). SWDGE (gpsimd): wfinB behind a stagger.
        agt_sb = agt_p.tile([128, KT, E], FP16, tag="agt", name="agt_sb")
        nc.sync.dma_start(agt_sb[:], agt_d[:, :, :])
        xt_sb = xt_p.tile([128, BL, KT, HW], FP16, tag="xt", name="xt_sb")
        xn_sb = xn_p.tile([128, 2 * BL, D], FP16, tag="xn", name="xn_sb")
        nc.scalar.dma_start(xt_sb[:, 0:2, :, :], xt_d[:, 0:2, :, :])
        nc.sync.dma_start(xn_sb[:, 0:4, :], xn_d[:, 0:4, :])
        for bp in range(1, 4):
            nc.sync.dma_start(xt_sb[:, 2 * bp:2 * bp + 2, :, :],
                              xt_d[:, 2 * bp:2 * bp + 2, :, :])
            nc.sync.dma_start(xn_sb[:, 4 * bp:4 * bp + 4, :],
                              xn_d[:, 4 * bp:4 * bp + 4, :])

        negb_sb = small_p.tile([128, E], FP32, tag="negb", name="negb_sb")
        nc.scalar.dma_start(negb_sb[:], negb_d[:, :])
        mask_sb = small_p.tile([128, C, EG], FP16, tag="mask", name="mask_sb")
        nc.scalar.dma_start(mask_sb[:], mask_d[:, :, :])
        sel3_sb = small_p.tile([128, BL], FP16, tag="sel3", name="sel3_sb")
        nc.scalar.dma_start(sel3_sb[:], sel3_d[:, :])
        # Pre-load the ACT op table (~1.3us, one-time) during the DMA phase
        # so the first real nc.scalar.copy doesn't stall the gt pipeline.
        act_warm = small_p.tile([128, 8], FP16, tag="act_warm",
                                name="act_warm")
        nc.scalar.copy(act_warm[:], warm_w[:, 0:8])

        # wfinA rides the SP ring right after the xt/xn stream (no dep --
        # it transfers while mm1/mm2 compute).
        wfin_sb = wfin_p.tile([128, 2, KT, NG // 2, C, EG], FP16, tag="wfin",
                              name="wfin_sb")
        nc.sync.dma_start(wfin_sb[:, 0, :, :, :, :], wfin_d[:, 0, :, :, :, :])

        # ---- mm1 + mm2, interleaved per batch pair so the PE stream is
        # ~2x denser than the xt/xn DMA stream (PE-bound, HAM stays warm).
        # mm1: S[t,e] = (x @ ag_w^T > -b), single fp16 pass.
        # mm2: G^T_b[d, e] = sum_h x[h,d] S[h,e];
        #      gt[dt][dp, g, b, el] = G^T_b[dt*128+dp, g*EG+el]
        sn_sb = [
            sn_p.tile([128, E], FP16, tag=f"sn{ch}", name=f"sn_sb{ch}")
            for ch in range(len(CHUNKS))
        ]
        gt_sb = [
            gt_p.tile([128, NG, BL, EG], FP16, tag=f"gt{dt}",
                      name=f"gt_sb{dt}")
            for dt in range(KT)
        ]
        th_insts = {}

        def emit_mm1(b):
            for ht in range(2):
                ch = 2 * b + ht
                _, _, h0, w = CHUNKS[ch]
                ps = ps1.tile([128, E], FP32, tag="ps1", name=f"ps1_{ch}")
                for kt in range(KT):
                    nc.tensor.matmul(
                        ps[0:w, :],
                        xt_sb[:, b, kt, h0:h0 + w],
                        agt_sb[:, kt, :],
                        start=(kt == 0),
                        stop=(kt == KT - 1),
                    )
                th_insts[ch] = nc.vector.tensor_tensor(
                    sn_sb[ch][0:w, :], ps[0:w, :], negb_sb[0:w, :],
                    mybir.AluOpType.is_gt,
                )

        def emit_mm2(b):
            for dt in range(KT):
                pg = ps2.tile([128, E], FP32, tag="ps2", name=f"ps2_{b}_{dt}")
                for ht in range(2):
                    ch = 2 * b + ht
                    w = CHUNKS[ch][3]
                    nc.tensor.matmul(
                        pg[:],
                        xn_sb[0:w, ch, dt * 128:(dt + 1) * 128],
                        sn_sb[ch][0:w, :],
                        start=(ht == 0),
                        stop=(ht == 1),
                    )
                if (b + dt) % 2 == 0:
                    nc.vector.tensor_copy(gt_sb[dt][:, :, b, :], pg[:])
                else:
                    nc.scalar.copy(gt_sb[dt][:, :, b, :], pg[:])

        for bp in range(4):
            emit_mm1(2 * bp)
            emit_mm1(2 * bp + 1)
            emit_mm2(2 * bp)
            emit_mm2(2 * bp + 1)
            if bp == 1:
                # wfinB on SWDGE once mm1 is underway; blocking the (other-
                # wise idle) gpsimd queue on this stagger is harmless.
                dma = nc.gpsimd.dma_start(wfin_sb[:, 1, :, :, :, :],
                                          wfin_d[:, 1, :, :, :, :])
                add_dep_helper(dma.ins, th_insts[2].ins,
                               reason="wfinB load after mm1 underway")

        # ---- final ----
        # mask (e-diagonal selection) is identical for every group, so it
        # commutes with the group sum: ALL cross-product matmuls accumulate
        # into one PSUM tile, masked once at the end.
        do_fin = "fin" in phases
        pm = psM.tile([128, C, EG], FP32, tag="psM", name="psM_t")
        if not do_fin:
            nc.vector.memset(pm[:], 0.0)
        nmm = KT * NG
        im = 0
        for g in range(NG if do_fin else 0):
            gh, gl = g // (NG // 2), g % (NG // 2)
            for dt in range(KT):
                nc.tensor.matmul(
                    pm[:],
                    gt_sb[dt][:, g, :, :],
                    wfin_sb[:, gh, dt, gl, :, :],
                    start=(im == 0),
                    stop=(im == nmm - 1),
                )
                im += 1
        msb = out_p.tile([128, C, EG], FP16, tag="msb", name="msb_t")
        nc.vector.tensor_tensor(
            msb[:], pm[:], mask_sb[:], mybir.AluOpType.mult
        )
        pf = psF.tile([BL, C, EG], FP32, tag="psf", name="psf_t")
        nc.tensor.matmul(pf[:], sel3_sb[:], msb[:], start=True, stop=True)

        # final tiny reduction over e + scale
        red_sb = out_p.tile([BL, C], FP32, tag="red", name="red_sb")
        nc.vector.tensor_reduce(
            red_sb[:], pf[:], mybir.AxisListType.X, mybir.AluOpType.add
        )
        out_sb = out_p.tile([BL, C], FP32, tag="out", name="out_sb")
        nc.vector.tensor_scalar_mul(out_sb[:], red_sb[:], 1.0 / (HW * E))
        nc.sync.dma_start(preds_o[:, :], out_sb[:])

    return nc


_program_cache = {}

CONFIG = {}


def _get_program(**kw):
    key = tuple(sorted(kw.items()))
    if key not in _program_cache:
        _program_cache[key] = build_program(**kw)
    return _program_cache[key]


def make_in_maps(x, ag_w, ag_b, lm_w, cfg=None):
    x = np.ascontiguousarray(np.asarray(x, dtype=np.float32))
    ag_w = np.asarray(ag_w, dtype=np.float32)
    ag_b = np.asarray(ag_b, dtype=np.float32)
    lm_w = np.asarray(lm_w, dtype=np.float32)

    agt = np.ascontiguousarray(
        ag_w.T.reshape(KT, 128, E).transpose(1, 0, 2).astype(np.float16)
    )
    negb = np.ascontiguousarray(
        np.broadcast_to(-ag_b[None, :], (128, E)).astype(np.float32)
    )
    # wfin[dp, gh, kt, gl, c, el] = lm_w[((gh*8+gl)*EG+el)*C+c, kt*128+dp]
    wfin = np.ascontiguousarray(
        lm_w.T.reshape(KT, 128, 2, NG // 2, EG, C)
        .transpose(1, 2, 0, 3, 5, 4)
        .astype(np.float16)
    )
    ep = np.arange(128) % EG
    mask = np.ascontiguousarray(
        (ep[:, None, None] == np.arange(EG)[None, None, :])
        * np.ones((128, C, EG), dtype=np.float16)
    )
    bidx = np.arange(128) // EG
    sel3 = (bidx[:, None] == np.arange(BL)[None, :]).astype(np.float16)

    common = {"agt": agt, "negb": negb, "wfin": wfin, "mask": mask,
              "sel3": sel3}
    in_maps = []
    for i in range(NCORES):
        xs = x[i * BL:(i + 1) * BL].reshape(T, D)
        m = dict(common)
        # xn_pk[p, (b,ht), d]: 128-row chunk + zero-padded 68-row tail chunk
        xn_pk = np.zeros((128, 2 * BL, D), dtype=np.float16)
        xsb = xs.reshape(BL, HW, D).astype(np.float16)
        for b in range(BL):
            xn_pk[:, 2 * b, :] = xsb[b, 0:128, :]
            xn_pk[0:HW - 128, 2 * b + 1, :] = xsb[b, 128:HW, :]
        m["xn"] = np.ascontiguousarray(xn_pk)
        # xt[dp, b, kt, h] = xs[b*HW+h, kt*128+dp]
        m["xt"] = np.ascontiguousarray(
            xs.T.reshape(KT, 128, BL, HW).transpose(1, 2, 0, 3)
            .astype(np.float16)
        )
        in_maps.append(m)
    return in_maps


def kernel(x, ag_w, ag_b, lm_w):
    in_maps = make_in_maps(x, ag_w, ag_b, lm_w)
    nc = _get_program()
    res = run_bass_kernel_spmd(nc, in_maps, core_ids=list(range(NCORES)))
    preds = np.concatenate(
        [res.results[i]["preds_o"] for i in range(NCORES)], axis=0
    )
    return np.ascontiguousarray(preds.astype(np.float32))
